# revision 1
# baseline (speedup 1.0000x reference)
"""Multi-head attention Trainium2 Bass kernel.

Problem: B=8, N=2048, C=768, H=12 heads, D=64 head dim.
  qkv = x @ w_qkv.T          -> [B, N, 3C]
  per head: softmax(q k^T / sqrt(D)) @ v
  y = attn_out @ w_proj.T + b_proj

Sharding: data parallel over batch — one batch element per NeuronCore (8 cores).

Per-core layout strategy (everything "transposed", feature-major):
  xT/w_qkvT/w_projT arrive pre-transposed from the host (free in numpy)
  qkvT [F, N] = W_qkv^T-stationary matmuls over xT   (F = 3C = 2304)
  S^T  [nk, nq] per head = kT-tile-stationary vs qT moving -> the softmax
       denominator comes from a ones-column appended to V in the A@V matmul
       (row 64 of the AV psum accumulates sum(exp(s))).
  exp via ScalarE (scale=1/8 folded in, no max subtraction: |scores| <~ 2.5)
  aT   [C, N] normalized attention output, fed as lhsT to the proj matmul.

Fully fused: each head pair's q/k/v is produced on-chip (w_qkvT f-tile
slices and xT chunks streamed from DRAM, no qkvT scratch round-trip); those
matmuls are dependency-free PE filler under the ScalarE exp chain, leaving
the kernel PE-bound at ~99% duty. The two heads of a pair occupy SBUF
partitions 0-63 / 64-127, and their S^T matmuls are interleaved per nk-tile
so adjacent instructions hit disjoint PE row groups (hardware overlaps the
two K=64 streams). Softmax normalization uses gpsimd partition_broadcast;
projection shares the attention scope and borrows the idle qkv psum pool.

All matmuls run in float32r (~1 cycle/row at free dim >= 256, rel err ~2e-4).
"""

import numpy as np

import concourse.bass as bass
import concourse.mybir as mybir
import concourse.tile as tile
from concourse import bacc
from concourse.bass_utils import run_bass_kernel_spmd
from concourse.masks import make_identity

B, N, C, H = 8, 2048, 768, 12
D = C // H            # 64
F = 3 * C             # 2304
NT = N // 128         # 16 seq tiles
CT = C // 128         # 6 channel tiles
FT = F // 128         # 18 qkv-feature tiles
NQ = 512              # query-chunk width (1 psum bank of fp32)
NCH = N // NQ         # 4 chunks
SCALE = float(D) ** -0.5

FP32 = mybir.dt.float32
FP32R = mybir.dt.float32r
EXP = mybir.ActivationFunctionType.Exp

_CACHED_NC = None


def _bc_ap(dram_ap, parts):
    """Partition-broadcast a 1-D DRAM AP to [parts, len] via stride-0."""
    return bass.AP(
        tensor=dram_ap.tensor,
        offset=dram_ap.offset,
        ap=[[0, parts]] + [list(p) for p in dram_ap.ap],
    )


def build():
    # xT/w_qkvT/w_projT arrive pre-transposed (feature-major) from the host:
    # the layout change is free in numpy and removes every input transpose
    # (PE + ScalarE evict) from the device timeline.
    nc = bacc.Bacc()
    x = nc.dram_tensor("xT", [C, N], FP32, kind="ExternalInput")
    w_qkv = nc.dram_tensor("w_qkvT", [C, F], FP32, kind="ExternalInput")
    w_proj = nc.dram_tensor("w_projT", [C, C], FP32, kind="ExternalInput")
    b_proj = nc.dram_tensor("b_proj", [C], FP32, kind="ExternalInput")
    y = nc.dram_tensor("y", [N, C], FP32, kind="ExternalOutput")
    aT_d = nc.dram_tensor("aT_scratch", [C, N], FP32R)

    xr = x[:, :].bitcast(FP32R)
    wqr = w_qkv[:, :].bitcast(FP32R)
    wpr = w_proj[:, :].bitcast(FP32R)

    lp = nc.allow_low_precision("float32r psum accumulation is fp32-width")
    lp.__enter__()
    with tile.TileContext(nc) as tc:
        const_cm = tc.tile_pool(name="const", bufs=1)
        const = const_cm.__enter__()
        ident_f = const.tile([128, 128], FP32)
        make_identity(nc, ident_f)
        ident = const.tile([128, 128], FP32R)
        nc.vector.tensor_copy(ident, ident_f)
        ones_row_f = const.tile([1, D], FP32)
        nc.vector.memset(ones_row_f, 1.0)
        ones_row = const.tile([1, D], FP32R)
        nc.vector.tensor_copy(ones_row, ones_row_f)
        ones_col = const.tile([128, NT, 1], FP32)
        nc.vector.memset(ones_col, 1.0)
        xr3 = xr.rearrange("(ko p) n -> p ko n", p=128)
        wqr3 = wqr.rearrange("(ko p) f -> p ko f", p=128)

        # ---------------- phase 2: attention, head pairs --------------------
        with tc.tile_pool(name="hpool", bufs=2) as hpool, \
             tc.tile_pool(name="spool", bufs=1) as spool, \
             tc.tile_pool(name="small", bufs=2) as small, \
             tc.tile_pool(name="psum_s", bufs=2, space="PSUM") as psum_s, \
             tc.tile_pool(name="psum_av", bufs=2, space="PSUM") as psum_av, \
             tc.tile_pool(name="psum_qkv", bufs=2, space="PSUM") as psum_qkv:

            for hp in range(H // 2):
                # produce this pair's q/k/v on-chip: stream the three w_qkvT
                # f-tiles {hp, 6+hp, 12+hp} and x chunks from DRAM; the qkv
                # matmuls are dependency-free PE filler under the exp chain.
                wqs = []
                for idx, m in enumerate((hp, CT + hp, 2 * CT + hp)):
                    w = hpool.tile(
                        [128, CT, 128], FP32R, tag=f"wq{idx}", name=f"wq{idx}",
                        bufs=1,
                    )
                    nc.sync.dma_start(
                        out=w, in_=wqr3[:, :, m * 128:(m + 1) * 128]
                    )
                    wqs.append(w)
                qTt = hpool.tile([128, N], FP32R, tag="qT")
                kTt = hpool.tile([128, N], FP32R, tag="kT")
                vTt = hpool.tile([128, N], FP32R, tag="vT")
                qkvts = (qTt, kTt, vTt)
                for j in range(NCH):
                    xc = hpool.tile([128, CT, NQ], FP32R, tag="xc", name="xc")
                    nc.sync.dma_start(
                        out=xc, in_=xr3[:, :, j * NQ:(j + 1) * NQ]
                    )
                    for idx in range(3):
                        ps = psum_qkv.tile([128, NQ], FP32, tag="qkvps", name="qkvps")
                        for k in range(CT):
                            nc.tensor.matmul(
                                ps,
                                wqs[idx][:, k, :],
                                xc[:, k, :],
                                start=(k == 0),
                                stop=(k == CT - 1),
                            )
                        nc.vector.tensor_copy(
                            qkvts[idx][:, j * NQ:(j + 1) * NQ], ps
                        )
                vaugs = []
                for a in range(2):
                    vaug = hpool.tile([128, NT, D + 1], FP32R, tag=f"vaug{a}")
                    nc.vector.tensor_copy(vaug[:, :, D:D + 1], ones_col)
                    vaugs.append(vaug)
                # A/B transposes interleaved per tile: adjacent PE
                # instructions hit disjoint row groups (0-63 / 64-127)
                for t0 in range(0, NT, 8):
                    pts = [
                        psum_av.tile(
                            [128, 8, D], FP32R, tag="av", name=f"pt{a}"
                        )
                        for a in range(2)
                    ]
                    for g in range(8):
                        t = t0 + g
                        for a in range(2):
                            lo = a * D
                            nc.tensor.transpose(
                                pts[a][:, g, :],
                                vTt[lo:lo + D, t * 128:(t + 1) * 128],
                                ident[lo:lo + D, lo:lo + D],
                            )
                    for a in range(2):
                        nc.vector.tensor_copy(
                            vaugs[a][:, t0:t0 + 8, 0:D], pts[a]
                        )

                # nk-tile group sizes: 3-bank psum tiles double-buffered so
                # ScalarE exp(g) overlaps the S^T matmuls of g+1.
                GROUPS = (2, 2, 2, 2, 2, 2, 2, 2)
                for j in range(NCH):
                    expSs = [
                        spool.tile(
                            [128, NT, NQ], FP32R,
                            tag=f"expS{a}", name=f"expS{a}",
                        )
                        for a in range(2)
                    ]
                    t = 0
                    for gsz in GROUPS:
                        # the two heads' matmuls are interleaved per nk-tile:
                        # adjacent MMs target disjoint PE row groups
                        # (partitions 0-63 / 64-127) and overlap in the array
                        sps_ab = [
                            psum_s.tile(
                                [128, 2, NQ], FP32, tag=f"sps{a}",
                                name=f"sps{a}", bufs=1,
                            )
                            for a in range(2)
                        ]
                        for u in range(gsz):
                            for a in range(2):
                                lo = a * D
                                nc.tensor.matmul(
                                    sps_ab[a][:, u, :],
                                    kTt[lo:lo + D, (t + u) * 128:(t + u + 1) * 128],
                                    qTt[lo:lo + D, j * NQ:(j + 1) * NQ],
                                    start=True,
                                    stop=True,
                                )
                        for a in range(2):
                            nc.scalar.activation(
                                out=expSs[a][:, t:t + gsz, :],
                                in_=sps_ab[a][:, 0:gsz, :],
                                func=EXP,
                                scale=SCALE,
                            )
                        t += gsz
                    for a in range(2):
                        h = 2 * hp + a
                        av = psum_av.tile([D + 1, NQ], FP32, tag="av")
                        for t in range(NT):
                            nc.tensor.matmul(
                                av,
                                vaugs[a][:, t, :],
                                expSs[a][:, t, :],
                                start=(t == 0),
                                stop=(t == NT - 1),
                            )
                        recip = small.tile([1, NQ], FP32, tag="recip")
                        nc.vector.reciprocal(recip, av[D:D + 1, :])
                        bc_sb = small.tile([D, NQ], FP32, tag="bc_sb")
                        nc.gpsimd.partition_broadcast(bc_sb, recip)
                        aTt = small.tile([D, NQ], FP32R, tag="aT_sb")
                        nc.vector.tensor_mul(aTt, av[0:D, :], bc_sb)
                        nc.sync.dma_start(
                            out=aT_d[h * D:(h + 1) * D, j * NQ:(j + 1) * NQ],
                            in_=aTt,
                        )

            # ---------- phase 3: output projection, inside the same scope.
            # proj psums borrow the qkv pool (idle once the last pair's
            # q/k/v are built), so proj matmuls fill the attention tail.
            bias_bc = small.tile([128, C], FP32, tag="bias", bufs=1)
            nc.gpsimd.dma_start(out=bias_bc, in_=_bc_ap(b_proj[:], 128))
            w_projT = small.tile([128, CT, C], FP32R, tag="wproj", bufs=1)
            nc.sync.dma_start(
                out=w_projT, in_=wpr.rearrange("(ko p) o -> p ko o", p=128)
            )
            NO = 384
            for i in range(NT):
                a_sb = small.tile([128, CT, 128], FP32R, tag="a_sb", bufs=2)
                nc.sync.dma_start(
                    out=a_sb,
                    in_=aT_d[:, i * 128:(i + 1) * 128].rearrange(
                        "(ko p) n -> p ko n", p=128
                    ),
                )
                for half in range(2):
                    ps = psum_qkv.tile([128, NO], FP32, tag="qkvps")
                    for k in range(CT):
                        nc.tensor.matmul(
                            ps,
                            a_sb[:, k, :],
                            w_projT[:, k, half * NO:(half + 1) * NO],
                            start=(k == 0),
                            stop=(k == CT - 1),
                        )
                    y_sb = small.tile([128, NO], FP32, tag="y_sb", bufs=2)
                    nc.vector.tensor_add(
                        y_sb, ps, bias_bc[:, half * NO:(half + 1) * NO]
                    )
                    nc.sync.dma_start(
                        out=y[i * 128:(i + 1) * 128, half * NO:(half + 1) * NO],
                        in_=y_sb,
                    )
        const_cm.__exit__(None, None, None)
    lp.__exit__(None, None, None)

    nc.finalize()
    return nc


def get_nc():
    global _CACHED_NC
    if _CACHED_NC is None:
        _CACHED_NC = build()
    return _CACHED_NC


LAST_RESULT = None


def kernel(x, w_qkv, w_proj, b_proj, **run_kwargs):
    x = np.ascontiguousarray(np.asarray(x, dtype=np.float32))
    w_qkv = np.ascontiguousarray(np.asarray(w_qkv, dtype=np.float32))
    w_proj = np.ascontiguousarray(np.asarray(w_proj, dtype=np.float32))
    b_proj = np.ascontiguousarray(np.asarray(b_proj, dtype=np.float32))
    assert x.shape == (B, N, C)

    nc = get_nc()
    w_qkvT = np.ascontiguousarray(w_qkv.T)
    w_projT = np.ascontiguousarray(w_proj.T)
    in_maps = [
        {
            "xT": np.ascontiguousarray(x[i].T),
            "w_qkvT": w_qkvT,
            "w_projT": w_projT,
            "b_proj": b_proj,
        }
        for i in range(B)
    ]
    res = run_bass_kernel_spmd(nc, in_maps, list(range(B)), **run_kwargs)
    global LAST_RESULT
    LAST_RESULT = res
    out = np.stack([res.results[i]["y"] for i in range(B)], axis=0)
    return out


if __name__ == "__main__":
    rng = np.random.default_rng(0)
    x = rng.standard_normal((B, N, C), dtype=np.float32)
    w_qkv = (rng.standard_normal((F, C)) * 0.02).astype(np.float32)
    w_proj = (rng.standard_normal((C, C)) * 0.02).astype(np.float32)
    b_proj = (rng.standard_normal((C,)) * 0.02).astype(np.float32)
    out = kernel(x=x, w_qkv=w_qkv, w_proj=w_proj, b_proj=b_proj)
    print("out", out.shape, out.dtype, float(np.abs(out).max()))



# revision 2
# speedup vs baseline: 1.3745x; 1.3745x over previous
"""Multi-head attention Trainium2 Bass kernel, v7.

B=8, N=2048, C=768, H=12, D=64. Data-parallel over batch: 1 element/core.

Per-core pipeline:
  QKV q,k  : fp8e4 DoubleRow residual matmuls.  The 1/16 residual scale is
             pre-baked into host copies of the weights (w_hi/16, w_lo/16), so
             the 9 DR matmuls accumulate the full-precision result into ONE
             psum chain; the evacuation is a single-input downcast that can run
             on either ScalarE (activation Copy) or DVE (tensor_scalar).
  V        : same residual-DR trick, key-major -> fp16 vaug [128, kt, 130]
             (two heads' 64+1 slots; ones col = softmax denominator).
  S^T      : fp8 DoubleRow, D=64 split across the two DR slots (d = 2p+i on 32
             partitions); 0.5 cycles/row.  q8/k8 reach [32,2,N] via an
             SBUF->SBUF DMA fold.
  exp      : three lanes, assigned per S-tile (pair of ktiles):
               ScalarE: true Exp -> fp16
               DVE:     Schraudolph fp16 magic (one fp32 mult-add; low 2 bytes
                        of each fp32 = the fp16 weight, read at stride 2)
               GPSIMD:  same magic trick on a DMA-staged SBUF copy of the
                        scores (Pool has no PSUM port).
  AV       : q-major fp16-moving matmuls; per-qb sequential 16-step chains into
             one packed psum bank [128, 4, 65(pad 128)].
  norm     : DVE reciprocal + scalar_tensor_tensor broadcast-mult -> fp16 a2;
             aT via DMA xbar transpose (no PE, no psum).
  proj     : fp16 matmuls; ScalarE Identity applies 1/(SW*SP) + per-partition
             bias; yT [C, N] fp32 out (host transposes).
"""

import math

import numpy as np
import ml_dtypes

import concourse.bass as bass
import concourse.mybir as mybir
import concourse.tile as tile
from concourse import bacc
from concourse.bass_utils import run_bass_kernel_spmd

B, N, C, H = 8, 2048, 768, 12
D = C // H            # 64
CT = C // 128         # 6 channel tiles
NQ = 512              # query chunk (1 psum bank fp32)
NCH = N // NQ         # 4
NKT = N // 128        # 16 key tiles

SW = 32.0             # host scale on w_qkv
SP = 32.0             # host scale on w_proj
SCALE = float(D) ** -0.5
S_SCALE = SCALE / (SW * SW)        # exp scale on raw q8.k8 psum scores
LN2 = math.log(2.0)
A16 = 1024.0 * S_SCALE / LN2       # fp16 magic slope
B16 = 15301.5  # centered fp16 exponent bias (+.5 for trunc converts)

# Per-head-parity exp-lane assignment over the 8 S-tiles (pairs of ktiles).
# Act is the faster exp engine (0.83 vs 1.04 ns/row): give it the bigger share.
LANES = (
    "AADADADA",   # head parity 0
    "DADADAAD",   # head parity 1
)
# evac lane for q/k f-major downcasts and v downcasts, by slice parity
QK_EVAC = ("A", "D")
V_EVAC = ("D", "A")

FP32 = mybir.dt.float32
FP16 = mybir.dt.float16
F8 = mybir.dt.float8e4
EXP = mybir.ActivationFunctionType.Exp
IDENT = mybir.ActivationFunctionType.Identity
COPY = mybir.ActivationFunctionType.Copy
MULT = mybir.AluOpType.mult
ADD = mybir.AluOpType.add
DR = mybir.MatmulPerfMode.DoubleRow

F8NP = ml_dtypes.float8_e4m3

_CACHED_NC = None


def _ap(base, free_dims):
    """AP with base's partition dim and explicit [stride, count] free dims."""
    return bass.AP(
        tensor=base.tensor,
        offset=base.offset,
        ap=[list(base.ap[0])] + [list(d) for d in free_dims],
    )


def build():
    nc = bacc.Bacc()
    x_hi = nc.dram_tensor("x_hi", [128, CT, N], F8, kind="ExternalInput")
    x_lo = nc.dram_tensor("x_lo", [128, CT, N], F8, kind="ExternalInput")
    w_hi = nc.dram_tensor("w_hi", [128, CT, 3 * C], F8, kind="ExternalInput")
    w_h16 = nc.dram_tensor("w_h16", [128, CT, 3 * C], F8, kind="ExternalInput")
    w_l16 = nc.dram_tensor("w_l16", [128, CT, 3 * C], F8, kind="ExternalInput")
    wp = nc.dram_tensor("wp", [128, CT, C], FP16, kind="ExternalInput")
    b2d = nc.dram_tensor("b2d", [128, CT], FP32, kind="ExternalInput")
    yT = nc.dram_tensor("yT", [C, N], FP32, kind="ExternalOutput")

    lp = nc.allow_low_precision("fp8/fp16 matmuls with fp32 psum accumulation")
    lp.__enter__()
    with tile.TileContext(nc) as tc:
        with tc.tile_pool(name="big", bufs=1) as big, \
             tc.tile_pool(name="fmp", bufs=2) as fmp, \
             tc.tile_pool(name="e16p", bufs=9) as e16p, \
             tc.tile_pool(name="e32p", bufs=9) as e32p, \
             tc.tile_pool(name="small", bufs=2) as small, \
             tc.tile_pool(name="ps2p", bufs=3, space="PSUM") as ps2p, \
             tc.tile_pool(name="psavp", bufs=2, space="PSUM") as psavp:

            # ---- persistent inputs -------------------------------------
            # loaded per ct-pair so the first DR chains start early
            xh = big.tile([128, CT, N], F8)
            xl = big.tile([128, CT, N], F8)
            wh = big.tile([128, CT, 3 * C], F8)
            wh16 = big.tile([128, CT, 3 * C], F8)
            wl16 = big.tile([128, CT, 3 * C], F8)
            for t in range(3):
                ts = slice(2 * t, 2 * t + 2)
                nc.sync.dma_start(out=wh[:, ts, :], in_=w_hi[:, ts, :])
                nc.sync.dma_start(out=xh[:, ts, :], in_=x_hi[:, ts, :])
                nc.sync.dma_start(out=xl[:, ts, :], in_=x_lo[:, ts, :])
                nc.sync.dma_start(out=wh16[:, ts, :], in_=w_h16[:, ts, :])
                nc.sync.dma_start(out=wl16[:, ts, :], in_=w_l16[:, ts, :])
            wpt = big.tile([128, CT, C], FP16)
            nc.sync.dma_start(out=wpt, in_=wp[:, :, :])
            bias = big.tile([128, CT], FP32)
            nc.sync.dma_start(out=bias, in_=b2d[:, :])

            qg = [big.tile([128, 2, N], F8, name=f"qg{g}") for g in range(4)]
            kg = [big.tile([128, 2, N], F8, name=f"kg{g}") for g in range(4)]
            vaug = [
                big.tile([128, NKT, 130], FP16, name=f"vaug{p}") for p in range(6)
            ]
            for p in range(6):
                nc.gpsimd.memset(vaug[p][:, :, 64:65], 1.0)
                nc.gpsimd.memset(vaug[p][:, :, 129:130], 1.0)
            aT = big.tile([128, CT, N], FP16)

            # residual-DR chain: 9 matmuls into one psum, full precision.
            # pairs: (hi, hi), (hi16, lo), (lo16, hi) on (weights, x) — caller
            # passes the already-matched (lhsT, rhs) AP pairs.
            def dr_chain(ps_out, pairs):
                k = 0
                for (lt, lsl), (rt, rsl) in pairs:
                    for t in range(3):
                        nc.tensor.matmul(
                            ps_out,
                            lt[:, 2 * t:2 * t + 2, lsl],
                            rt[:, 2 * t:2 * t + 2, rsl],
                            start=(k == 0), stop=(k == 8), perf_mode=DR,
                        )
                        k += 1

            def evac(lane, out, in_):
                if lane == "A":
                    nc.scalar.activation(out=out, in_=in_, func=COPY, scale=1.0)
                else:
                    nc.vector.tensor_copy(out, in_)

            # ---- phase A pieces ----------------------------------------
            def emit_qk_slice(hp, ch, fms):
                n0 = ch * NQ
                nsl = slice(n0, n0 + NQ)
                ps = ps2p.tile([128, 2, NQ], FP32, tag="ps2", name="psqk")
                for side in (0, 1):
                    f0 = 128 * (hp + 6 * side)
                    fsl = slice(f0, f0 + 128)
                    dr_chain(
                        ps[:, side, :],
                        (((wh, fsl), (xh, nsl)),
                         ((wh16, fsl), (xl, nsl)),
                         ((wl16, fsl), (xh, nsl))),
                    )
                for side in (0, 1):
                    evac(QK_EVAC[(ch + side) % 2],
                         fms[side][:, n0:n0 + NQ], ps[:, side, :])

            def emit_v_slice(hp, ch):
                vf0 = 1536 + 128 * hp
                vsl = slice(vf0, vf0 + 128)
                ps = ps2p.tile([128, 2, NQ], FP32, tag="ps2", name="psv")
                for i in range(4):
                    kt = 4 * ch + i
                    n0 = kt * 128
                    nsl = slice(n0, n0 + 128)
                    dr_chain(
                        ps[:, i // 2, 128 * (i % 2):128 * (i % 2) + 128],
                        (((xh, nsl), (wh, vsl)),
                         ((xl, nsl), (wh16, vsl)),
                         ((xh, nsl), (wl16, vsl))),
                    )
                # psum [128, bank, chain(2), head(2), 64] -> vaug rows
                for b in (0, 1):
                    vrow = vaug[hp][:, 4 * ch + 2 * b, :]
                    out = _ap(vrow, [[130, 2], [65, 2], [1, 64]])
                    src = _ap(ps[:, b, :], [[128, 2], [64, 2], [1, 64]])
                    evac(V_EVAC[(ch + b) % 2], out, src)

            def new_fm():
                fq = fmp.tile([128, N], F8, tag="fmq", name="fq")
                fk = fmp.tile([128, N], F8, tag="fmk", name="fk")
                return fq, fk

            def emit_a_slice(hp, ch, fms):
                emit_qk_slice(hp, ch, fms)
                emit_v_slice(hp, ch)

            def emit_rearrange_ch(hp, ch, fms):
                # fold [64, NQ] f-major chunk into [32, 2, NQ] (d = 2p + i)
                n0 = ch * NQ
                for side, grps in ((0, qg), (1, kg)):
                    for hi in (0, 1):
                        h = 2 * hp + hi
                        g, q4 = h // 3, h % 3
                        nc.sync.dma_start(
                            out=grps[g][32 * q4:32 * q4 + 32, :, n0:n0 + NQ],
                            in_=fms[side][64 * hi:64 * hi + 64, n0:n0 + NQ],
                        )

            # ---- phase B: S+exp for head k runs while head k-1's AV/norm
            # retires, so the exp engines always have fresh psums ------------
            def emit_s_exp(hp, ch, hi):
                n0 = ch * NQ
                h = 2 * hp + hi
                g, q4 = h // 3, h % 3
                p0 = 32 * q4
                qs = qg[g][p0:p0 + 32, :, n0:n0 + NQ]
                lanes = LANES[hi]
                ets = []
                for st in range(8):
                    ps = ps2p.tile([128, 2, NQ], FP32, tag="ps2", name="sps")
                    for tt in (0, 1):
                        kt = 2 * st + tt
                        nc.tensor.matmul(
                            ps[:, tt, :],
                            kg[g][p0:p0 + 32, :, kt * 128:kt * 128 + 128],
                            qs,
                            start=True, stop=True, perf_mode=DR,
                        )
                    if lanes[st] == "A":
                        et = e16p.tile([128, 2, NQ], FP16, tag="e16", name="e16")
                        nc.scalar.activation(
                            out=et, in_=ps, func=EXP, scale=S_SCALE
                        )
                        ets.append((et, False))
                    else:
                        # fp32 -> uint16 convert IS the magic: i16 lands as the
                        # fp16 bit pattern of ~exp(s)
                        et = e32p.tile([128, 2, NQ], mybir.dt.uint16,
                                       tag="e32", name="e32")
                        nc.vector.tensor_scalar(
                            out=et, in0=ps, scalar1=A16, scalar2=B16,
                            op0=MULT, op1=ADD,
                        )
                        ets.append((et, True))
                return (hp, ch, hi, ets)

            a2map = {}

            def emit_av_norm(ctx):
                hp, ch, hi, ets = ctx
                n0 = ch * NQ
                if hi == 0:
                    a2map[(hp, ch)] = small.tile(
                        [128, 4, 128], FP16, tag="a2", name="a2", bufs=3
                    )
                a2 = a2map[(hp, ch)]
                av = psavp.tile([128, 4, 128], FP32, tag="av", name="av")
                for qb in range(4):
                    for st in range(8):
                        et, magic = ets[st]
                        for tt in (0, 1):
                            kt = 2 * st + tt
                            if magic:
                                l = et.bitcast(FP16)[:, tt, qb * 128:qb * 128 + 128]
                            else:
                                l = et[:, tt, qb * 128:qb * 128 + 128]
                            nc.tensor.matmul(
                                av[:, qb, 0:65],
                                l,
                                vaug[hp][:, kt, 65 * hi:65 * hi + 65],
                                start=(st == 0 and tt == 0),
                                stop=(st == 7 and tt == 1),
                            )
                recp = small.tile([128, 4], FP32, tag="recp", name="recp")
                nc.vector.reciprocal(recp, av[:, :, 64])
                nc.vector.scalar_tensor_tensor(
                    out=a2[:, :, 64 * hi:64 * hi + 64],
                    in0=av[:, :, 0:64],
                    scalar=1.0,
                    in1=_ap(recp, [[1, 4], [0, 64]]),
                    op0=MULT, op1=MULT,
                )
                if hi == 1:
                    nc.sync.dma_start_transpose(
                        out=aT[:, hp, n0:n0 + NQ].rearrange(
                            "p (qb q) -> p qb q", qb=4
                        ),
                        in_=a2.rearrange("p qb d -> p (qb d)"),
                    )
                    del a2map[(hp, ch)]

            # ---- phase C ------------------------------------------------
            def emit_c():
                for ot in range(6):
                    for ch in range(NCH):
                        n0 = ch * NQ
                        ps = ps2p.tile([128, 2, NQ], FP32, tag="ps2", name="cps")
                        for ct in range(CT):
                            nc.tensor.matmul(
                                ps[:, 0, :],
                                wpt[:, ct, 128 * ot:128 * ot + 128],
                                aT[:, ct, n0:n0 + NQ],
                                start=(ct == 0), stop=(ct == CT - 1),
                            )
                        ysb = small.tile([128, NQ], FP32, tag="ysb", name="ysb")
                        nc.scalar.activation(
                            out=ysb, in_=ps[:, 0, :], func=IDENT,
                            scale=1.0 / (SW * SP), bias=bias[:, ot:ot + 1],
                        )
                        nc.sync.dma_start(
                            out=yT[128 * ot:128 * ot + 128, n0:n0 + NQ], in_=ysb
                        )

            # ---- emission: A slices pipelined one pair ahead of B, AV
            # blocks deferred one head behind their S+exp ------------------
            fms = new_fm()
            for ch in range(NCH):
                emit_a_slice(0, ch, fms)
                emit_rearrange_ch(0, ch, fms)
            pend = None
            for hp in range(6):
                nfms = new_fm() if hp < 5 else None
                for ch in range(NCH):
                    for hi in (0, 1):
                        ctx = emit_s_exp(hp, ch, hi)
                        if pend is not None:
                            emit_av_norm(pend)
                        pend = ctx
                        if hp < 5:
                            if hi == 0:
                                emit_qk_slice(hp + 1, ch, nfms)
                            else:
                                emit_v_slice(hp + 1, ch)
                                emit_rearrange_ch(hp + 1, ch, nfms)
            emit_av_norm(pend)
            emit_c()
    lp.__exit__(None, None, None)

    nc.finalize()
    return nc


def get_nc():
    global _CACHED_NC
    if _CACHED_NC is None:
        _CACHED_NC = build()
    return _CACHED_NC


def _prep_shared(w_qkv, w_proj, b_proj):
    wq = (w_qkv.astype(np.float64) * SW).astype(np.float32)
    w_hi = wq.astype(F8NP)
    w_lo = ((wq - w_hi.astype(np.float32)) * 16.0).astype(F8NP)
    w_h16 = (w_hi.astype(np.float32) / 16.0).astype(F8NP)
    w_l16 = (w_lo.astype(np.float32) / 16.0).astype(F8NP)

    def lay_w(a):
        return np.ascontiguousarray(a.T.reshape(CT, 128, 3 * C).transpose(1, 0, 2))
    wpm = (w_proj.astype(np.float64) * SP).astype(np.float16)
    wp_l = np.ascontiguousarray(wpm.T.reshape(CT, 128, C).transpose(1, 0, 2))
    b2d = np.ascontiguousarray(b_proj.reshape(CT, 128).T.astype(np.float32))
    return lay_w(w_hi), lay_w(w_h16), lay_w(w_l16), wp_l, b2d


def _prep_x(xi):
    xs = np.ascontiguousarray(xi.T.reshape(CT, 128, N).transpose(1, 0, 2))
    x_hi = xs.astype(F8NP)
    x_lo = ((xs - x_hi.astype(np.float32)) * 16.0).astype(F8NP)
    return x_hi, x_lo


LAST_RESULT = None


def kernel(x, w_qkv, w_proj, b_proj, **run_kwargs):
    x = np.ascontiguousarray(np.asarray(x, dtype=np.float32))
    w_qkv = np.ascontiguousarray(np.asarray(w_qkv, dtype=np.float32))
    w_proj = np.ascontiguousarray(np.asarray(w_proj, dtype=np.float32))
    b_proj = np.ascontiguousarray(np.asarray(b_proj, dtype=np.float32))
    assert x.shape == (B, N, C)

    nc = get_nc()
    w_hi, w_h16, w_l16, wp_l, b2d = _prep_shared(w_qkv, w_proj, b_proj)
    in_maps = []
    for i in range(B):
        x_hi, x_lo = _prep_x(x[i])
        in_maps.append({
            "x_hi": x_hi, "x_lo": x_lo,
            "w_hi": w_hi, "w_h16": w_h16, "w_l16": w_l16,
            "wp": wp_l, "b2d": b2d,
        })
    res = run_bass_kernel_spmd(nc, in_maps, list(range(B)), **run_kwargs)
    global LAST_RESULT
    LAST_RESULT = res
    out = np.stack(
        [np.ascontiguousarray(res.results[i]["yT"].T) for i in range(B)], axis=0
    )
    return out


if __name__ == "__main__":
    rng = np.random.default_rng(0)
    x = rng.standard_normal((B, N, C), dtype=np.float32)
    w_qkv = (rng.standard_normal((3 * C, C)) * 0.02).astype(np.float32)
    w_proj = (rng.standard_normal((C, C)) * 0.02).astype(np.float32)
    b_proj = (rng.standard_normal((C,)) * 0.02).astype(np.float32)
    out = kernel(x=x, w_qkv=w_qkv, w_proj=w_proj, b_proj=b_proj)
    print("out", out.shape, out.dtype, float(np.abs(out).max()))


# revision 3
# speedup vs baseline: 1.4462x; 1.0521x over previous
"""Multi-head attention Trainium2 Bass kernel, v7.

B=8, N=2048, C=768, H=12, D=64. Data-parallel over batch: 1 element/core.

Per-core pipeline:
  QKV q,k  : fp8e4 DoubleRow residual matmuls.  The 1/16 residual scale is
             pre-baked into host copies of the weights (w_hi/16, w_lo/16), so
             the 9 DR matmuls accumulate the full-precision result into ONE
             psum chain; the evacuation is a single-input downcast that can run
             on either ScalarE (activation Copy) or DVE (tensor_scalar).
  V        : same residual-DR trick, key-major -> fp16 vaug [128, kt, 130]
             (two heads' 64+1 slots; ones col = softmax denominator).
  S^T      : fp8 DoubleRow, D=64 split across the two DR slots (d = 2p+i on 32
             partitions); 0.5 cycles/row.  q8/k8 reach [32,2,N] via an
             SBUF->SBUF DMA fold.
  exp      : three lanes, assigned per S-tile (pair of ktiles):
               ScalarE: true Exp -> fp16
               DVE:     Schraudolph fp16 magic (one fp32 mult-add; low 2 bytes
                        of each fp32 = the fp16 weight, read at stride 2)
               GPSIMD:  same magic trick on a DMA-staged SBUF copy of the
                        scores (Pool has no PSUM port).
  AV       : q-major fp16-moving matmuls; per-qb sequential 16-step chains into
             one packed psum bank [128, 4, 65(pad 128)].
  norm     : DVE reciprocal + scalar_tensor_tensor broadcast-mult -> fp16 a2;
             aT via DMA xbar transpose (no PE, no psum).
  proj     : fp16 matmuls; ScalarE Identity applies 1/(SW*SP) + per-partition
             bias; yT [C, N] fp32 out (host transposes).
"""

import math

import numpy as np
import ml_dtypes

import concourse.bass as bass
import concourse.mybir as mybir
import concourse.tile as tile
from concourse import bacc
from concourse.bass_utils import run_bass_kernel_spmd

B, N, C, H = 8, 2048, 768, 12
D = C // H            # 64
CT = C // 128         # 6 channel tiles
NQ = 512              # query chunk (1 psum bank fp32)
NCH = N // NQ         # 4
NKT = N // 128        # 16 key tiles

SW = 32.0             # host scale on w_qkv
SP = 32.0             # host scale on w_proj
SCALE = float(D) ** -0.5
S_SCALE = SCALE / (SW * SW)        # exp scale on raw q8.k8 psum scores
LN2 = math.log(2.0)
A16 = 1024.0 * S_SCALE / LN2       # fp16 magic slope
B16 = 15301.5  # centered fp16 exponent bias (+.5 for trunc converts)

# Per-head-parity exp-lane assignment over the 8 S-tiles (pairs of ktiles).
# Act is the faster exp engine (0.83 vs 1.04 ns/row): give it the bigger share.
LANES = (
    "AADADADA",   # head parity 0
    "DADADAAD",   # head parity 1
)
# evac lane for q/k f-major downcasts and v downcasts, by slice parity
QK_EVAC = ("D", "D")
V_EVAC = ("A", "A")

FP32 = mybir.dt.float32
FP16 = mybir.dt.float16
F8 = mybir.dt.float8e4
EXP = mybir.ActivationFunctionType.Exp
IDENT = mybir.ActivationFunctionType.Identity
COPY = mybir.ActivationFunctionType.Copy
MULT = mybir.AluOpType.mult
ADD = mybir.AluOpType.add
DR = mybir.MatmulPerfMode.DoubleRow

F8NP = ml_dtypes.float8_e4m3

_CACHED_NC = None


def _ap(base, free_dims):
    """AP with base's partition dim and explicit [stride, count] free dims."""
    return bass.AP(
        tensor=base.tensor,
        offset=base.offset,
        ap=[list(base.ap[0])] + [list(d) for d in free_dims],
    )


def build():
    nc = bacc.Bacc()
    x_hi = nc.dram_tensor("x_hi", [128, CT, N], F8, kind="ExternalInput")
    x_lo = nc.dram_tensor("x_lo", [128, CT, N], F8, kind="ExternalInput")
    w_hi = nc.dram_tensor("w_hi", [128, CT, 3 * C], F8, kind="ExternalInput")
    w_lo = nc.dram_tensor("w_lo", [128, CT, 3 * C], F8, kind="ExternalInput")
    wp = nc.dram_tensor("wp", [128, CT, C], FP16, kind="ExternalInput")
    b2d = nc.dram_tensor("b2d", [128, CT], FP32, kind="ExternalInput")
    yT = nc.dram_tensor("yT", [C, N], FP32, kind="ExternalOutput")

    lp = nc.allow_low_precision("fp8/fp16 matmuls with fp32 psum accumulation")
    lp.__enter__()
    with tile.TileContext(nc) as tc:
        with tc.tile_pool(name="big", bufs=1) as big, \
             tc.tile_pool(name="fmp", bufs=2) as fmp, \
             tc.tile_pool(name="e16p", bufs=7) as e16p, \
             tc.tile_pool(name="e32p", bufs=8) as e32p, \
             tc.tile_pool(name="small", bufs=2) as small, \
             tc.tile_pool(name="ps2p", bufs=3, space="PSUM") as ps2p, \
             tc.tile_pool(name="psavp", bufs=2, space="PSUM") as psavp:

            # ---- persistent inputs -------------------------------------
            # loaded per ct-pair so the first DR chains start early
            xh = big.tile([128, CT, N], F8)
            xl = big.tile([128, CT, N], F8)
            wh = big.tile([128, CT, 3 * C], F8)
            wl = big.tile([128, CT, 3 * C], F8)
            wh16 = big.tile([128, CT, 3 * C], F8)
            xh16 = big.tile([128, CT, N], F8)
            for t in range(3):
                ts = slice(2 * t, 2 * t + 2)
                nc.sync.dma_start(out=wh[:, ts, :], in_=w_hi[:, ts, :])
                nc.sync.dma_start(out=xh[:, ts, :], in_=x_hi[:, ts, :])
                nc.sync.dma_start(out=xl[:, ts, :], in_=x_lo[:, ts, :])
                nc.sync.dma_start(out=wl[:, ts, :], in_=w_lo[:, ts, :])
            # derive the 1/16-prescaled operands on-chip (engines idle here)
            for t in range(3):
                ts = slice(2 * t, 2 * t + 2)
                nc.scalar.activation(out=wh16[:, ts, :], in_=wh[:, ts, :],
                                     func=COPY, scale=1.0 / 16.0)
                nc.vector.tensor_scalar(out=xh16[:, ts, :], in0=xh[:, ts, :],
                                        scalar1=1.0 / 16.0, scalar2=None,
                                        op0=MULT)
            wpt = big.tile([128, CT, C], FP16)
            nc.sync.dma_start(out=wpt, in_=wp[:, :, :])
            bias = big.tile([128, CT], FP32)
            nc.sync.dma_start(out=bias, in_=b2d[:, :])

            qg = [big.tile([128, 2, N], F8, name=f"qg{g}") for g in range(4)]
            kg = [big.tile([128, 2, N], F8, name=f"kg{g}") for g in range(4)]
            vaug = [
                big.tile([128, NKT, 130], FP16, name=f"vaug{p}") for p in range(6)
            ]
            for p in range(6):
                nc.gpsimd.memset(vaug[p][:, :, 64:65], 1.0)
                nc.gpsimd.memset(vaug[p][:, :, 129:130], 1.0)
            aT = big.tile([128, CT, N], FP16)

            # residual-DR chain: 9 matmuls into one psum, full precision.
            # pairs: (hi, hi), (hi16, lo), (lo16, hi) on (weights, x) — caller
            # passes the already-matched (lhsT, rhs) AP pairs.
            def dr_chain(ps_out, pairs):
                k = 0
                for (lt, lsl), (rt, rsl) in pairs:
                    for t in range(3):
                        nc.tensor.matmul(
                            ps_out,
                            lt[:, 2 * t:2 * t + 2, lsl],
                            rt[:, 2 * t:2 * t + 2, rsl],
                            start=(k == 0), stop=(k == 8), perf_mode=DR,
                        )
                        k += 1

            def evac(lane, out, in_):
                if lane == "A":
                    nc.scalar.activation(out=out, in_=in_, func=COPY, scale=1.0)
                else:
                    nc.vector.tensor_copy(out, in_)

            # ---- phase A pieces ----------------------------------------
            def emit_qk_side(hp, ch, side, fms):
                n0 = ch * NQ
                nsl = slice(n0, n0 + NQ)
                ps = ps2p.tile([128, 2, NQ], FP32, tag="ps2", name="psqk")
                f0 = 128 * (hp + 6 * side)
                fsl = slice(f0, f0 + 128)
                dr_chain(
                    ps[:, 0, :],
                    (((wh, fsl), (xh, nsl)),
                     ((wh16, fsl), (xl, nsl)),
                     ((wl, fsl), (xh16, nsl))),
                )
                evac(QK_EVAC[(ch + side) % 2],
                     fms[side][:, n0:n0 + NQ], ps[:, 0, :])

            def emit_v_half(hp, ch, j):
                # 2 key-tiles of this pair's V: kts {4ch+2j, 4ch+2j+1}
                vf0 = 1536 + 128 * hp
                vsl = slice(vf0, vf0 + 128)
                ps = ps2p.tile([128, 2, NQ], FP32, tag="ps2", name="psv")
                for i in (0, 1):
                    kt = 4 * ch + 2 * j + i
                    n0 = kt * 128
                    nsl = slice(n0, n0 + 128)
                    dr_chain(
                        ps[:, i, 0:128],
                        (((xh, nsl), (wh, vsl)),
                         ((xl, nsl), (wh16, vsl)),
                         ((xh16, nsl), (wl, vsl))),
                    )
                vrow = vaug[hp][:, 4 * ch + 2 * j, :]
                out = _ap(vrow, [[130, 2], [65, 2], [1, 64]])
                src = _ap(ps[:, 0, :], [[512, 2], [64, 2], [1, 64]])
                evac(V_EVAC[(ch + j) % 2], out, src)

            def new_fm():
                fq = fmp.tile([128, N], F8, tag="fmq", name="fq")
                fk = fmp.tile([128, N], F8, tag="fmk", name="fk")
                return fq, fk

            def emit_a_slice(hp, ch, fms):
                emit_qk_side(hp, ch, 0, fms)
                emit_v_half(hp, ch, 0)
                emit_qk_side(hp, ch, 1, fms)
                emit_v_half(hp, ch, 1)

            def emit_rearrange_ch(hp, ch, fms):
                # fold [64, NQ] f-major chunk into [32, 2, NQ] (d = 2p + i)
                n0 = ch * NQ
                for side, grps in ((0, qg), (1, kg)):
                    for hi in (0, 1):
                        h = 2 * hp + hi
                        g, q4 = h // 3, h % 3
                        nc.sync.dma_start(
                            out=grps[g][32 * q4:32 * q4 + 32, :, n0:n0 + NQ],
                            in_=fms[side][64 * hi:64 * hi + 64, n0:n0 + NQ],
                        )

            # ---- phase B: S+exp for head k runs while head k-1's AV/norm
            # retires, so the exp engines always have fresh psums ------------
            def emit_s_exp(hp, ch, hi):
                n0 = ch * NQ
                h = 2 * hp + hi
                g, q4 = h // 3, h % 3
                p0 = 32 * q4
                qs = qg[g][p0:p0 + 32, :, n0:n0 + NQ]
                lanes = LANES[hi]
                ets = []
                for st in range(8):
                    ps = ps2p.tile([128, 2, NQ], FP32, tag="ps2", name="sps")
                    for tt in (0, 1):
                        kt = 2 * st + tt
                        nc.tensor.matmul(
                            ps[:, tt, :],
                            kg[g][p0:p0 + 32, :, kt * 128:kt * 128 + 128],
                            qs,
                            start=True, stop=True, perf_mode=DR,
                        )
                    if lanes[st] == "A":
                        et = e16p.tile([128, 2, NQ], FP16, tag="e16", name="e16")
                        nc.scalar.activation(
                            out=et, in_=ps, func=EXP, scale=S_SCALE
                        )
                        ets.append((et, False))
                    else:
                        # fp32 -> uint16 convert IS the magic: i16 lands as the
                        # fp16 bit pattern of ~exp(s)
                        et = e32p.tile([128, 2, NQ], mybir.dt.uint16,
                                       tag="e32", name="e32")
                        nc.vector.tensor_scalar(
                            out=et, in0=ps, scalar1=A16, scalar2=B16,
                            op0=MULT, op1=ADD,
                        )
                        ets.append((et, True))
                return (hp, ch, hi, ets)

            a2map = {}

            def emit_av_norm(ctx):
                hp, ch, hi, ets = ctx
                n0 = ch * NQ
                if hi == 0:
                    a2map[(hp, ch)] = small.tile(
                        [128, 4, 128], FP16, tag="a2", name="a2", bufs=2
                    )
                a2 = a2map[(hp, ch)]
                av = psavp.tile([128, 4, 128], FP32, tag="av", name="av")
                for qb in range(4):
                    for st in range(8):
                        et, magic = ets[st]
                        for tt in (0, 1):
                            kt = 2 * st + tt
                            if magic:
                                l = et.bitcast(FP16)[:, tt, qb * 128:qb * 128 + 128]
                            else:
                                l = et[:, tt, qb * 128:qb * 128 + 128]
                            nc.tensor.matmul(
                                av[:, qb, 0:65],
                                l,
                                vaug[hp][:, kt, 65 * hi:65 * hi + 65],
                                start=(st == 0 and tt == 0),
                                stop=(st == 7 and tt == 1),
                            )
                recp = small.tile([128, 4], FP32, tag="recp", name="recp")
                nc.vector.reciprocal(recp, av[:, :, 64])
                nc.vector.scalar_tensor_tensor(
                    out=a2[:, :, 64 * hi:64 * hi + 64],
                    in0=av[:, :, 0:64],
                    scalar=1.0,
                    in1=_ap(recp, [[1, 4], [0, 64]]),
                    op0=MULT, op1=MULT,
                )
                if hi == 1:
                    nc.sync.dma_start_transpose(
                        out=aT[:, hp, n0:n0 + NQ].rearrange(
                            "p (qb q) -> p qb q", qb=4
                        ),
                        in_=a2.rearrange("p qb d -> p (qb d)"),
                    )
                    del a2map[(hp, ch)]

            # ---- phase C ------------------------------------------------
            def emit_c():
                for ot in range(6):
                    for ch in range(NCH):
                        n0 = ch * NQ
                        ps = ps2p.tile([128, 2, NQ], FP32, tag="ps2", name="cps")
                        for ct in range(CT):
                            nc.tensor.matmul(
                                ps[:, 0, :],
                                wpt[:, ct, 128 * ot:128 * ot + 128],
                                aT[:, ct, n0:n0 + NQ],
                                start=(ct == 0), stop=(ct == CT - 1),
                            )
                        ysb = e32p.tile([128, NQ], FP32, tag="e32", name="ysb")
                        nc.scalar.activation(
                            out=ysb, in_=ps[:, 0, :], func=IDENT,
                            scale=1.0 / (SW * SP), bias=bias[:, ot:ot + 1],
                        )
                        nc.sync.dma_start(
                            out=yT[128 * ot:128 * ot + 128, n0:n0 + NQ], in_=ysb
                        )

            # ---- emission: A slices pipelined one pair ahead of B, AV
            # blocks deferred one head behind their S+exp ------------------
            fms = new_fm()
            for ch in range(NCH):
                emit_a_slice(0, ch, fms)
                emit_rearrange_ch(0, ch, fms)
            pend = None
            for hp in range(6):
                nfms = new_fm() if hp < 5 else None
                for ch in range(NCH):
                    for hi in (0, 1):
                        ctx = emit_s_exp(hp, ch, hi)
                        if pend is not None:
                            emit_av_norm(pend)
                        pend = ctx
                        if hp < 5:
                            emit_qk_side(hp + 1, ch, hi, nfms)
                            emit_v_half(hp + 1, ch, hi)
                            if hi == 1:
                                emit_rearrange_ch(hp + 1, ch, nfms)
            emit_av_norm(pend)
            emit_c()
    lp.__exit__(None, None, None)

    nc.finalize()
    return nc


def get_nc():
    global _CACHED_NC
    if _CACHED_NC is None:
        _CACHED_NC = build()
    return _CACHED_NC


def _prep_shared(w_qkv, w_proj, b_proj):
    wq = (w_qkv.astype(np.float64) * SW).astype(np.float32)
    w_hi = wq.astype(F8NP)
    w_lo = ((wq - w_hi.astype(np.float32)) * 16.0).astype(F8NP)

    def lay_w(a):
        return np.ascontiguousarray(a.T.reshape(CT, 128, 3 * C).transpose(1, 0, 2))
    wpm = (w_proj.astype(np.float64) * SP).astype(np.float16)
    wp_l = np.ascontiguousarray(wpm.T.reshape(CT, 128, C).transpose(1, 0, 2))
    b2d = np.ascontiguousarray(b_proj.reshape(CT, 128).T.astype(np.float32))
    return lay_w(w_hi), lay_w(w_lo), wp_l, b2d


def _prep_x(xi):
    xs = np.ascontiguousarray(xi.T.reshape(CT, 128, N).transpose(1, 0, 2))
    x_hi = xs.astype(F8NP)
    x_lo = ((xs - x_hi.astype(np.float32)) * 16.0).astype(F8NP)
    return x_hi, x_lo


LAST_RESULT = None


def kernel(x, w_qkv, w_proj, b_proj, **run_kwargs):
    x = np.ascontiguousarray(np.asarray(x, dtype=np.float32))
    w_qkv = np.ascontiguousarray(np.asarray(w_qkv, dtype=np.float32))
    w_proj = np.ascontiguousarray(np.asarray(w_proj, dtype=np.float32))
    b_proj = np.ascontiguousarray(np.asarray(b_proj, dtype=np.float32))
    assert x.shape == (B, N, C)

    nc = get_nc()
    w_hi, w_lo_l, wp_l, b2d = _prep_shared(w_qkv, w_proj, b_proj)
    in_maps = []
    for i in range(B):
        x_hi, x_lo = _prep_x(x[i])
        in_maps.append({
            "x_hi": x_hi, "x_lo": x_lo,
            "w_hi": w_hi, "w_lo": w_lo_l,
            "wp": wp_l, "b2d": b2d,
        })
    res = run_bass_kernel_spmd(nc, in_maps, list(range(B)), **run_kwargs)
    global LAST_RESULT
    LAST_RESULT = res
    out = np.stack(
        [np.ascontiguousarray(res.results[i]["yT"].T) for i in range(B)], axis=0
    )
    return out


if __name__ == "__main__":
    rng = np.random.default_rng(0)
    x = rng.standard_normal((B, N, C), dtype=np.float32)
    w_qkv = (rng.standard_normal((3 * C, C)) * 0.02).astype(np.float32)
    w_proj = (rng.standard_normal((C, C)) * 0.02).astype(np.float32)
    b_proj = (rng.standard_normal((C,)) * 0.02).astype(np.float32)
    out = kernel(x=x, w_qkv=w_qkv, w_proj=w_proj, b_proj=b_proj)
    print("out", out.shape, out.dtype, float(np.abs(out).max()))


# revision 4
# speedup vs baseline: 1.4643x; 1.0125x over previous
"""Multi-head attention Trainium2 Bass kernel, v7.

B=8, N=2048, C=768, H=12, D=64. Data-parallel over batch: 1 element/core.

Per-core pipeline:
  QKV q,k  : fp8e4 DoubleRow residual matmuls.  The 1/16 residual scale is
             pre-baked into host copies of the weights (w_hi/16, w_lo/16), so
             the 9 DR matmuls accumulate the full-precision result into ONE
             psum chain; the evacuation is a single-input downcast that can run
             on either ScalarE (activation Copy) or DVE (tensor_scalar).
  V        : same residual-DR trick, key-major -> fp16 vaug [128, kt, 130]
             (two heads' 64+1 slots; ones col = softmax denominator).
  S^T      : fp8 DoubleRow, D=64 split across the two DR slots (d = 2p+i on 32
             partitions); 0.5 cycles/row.  q8/k8 reach [32,2,N] via an
             SBUF->SBUF DMA fold.
  exp      : three lanes, assigned per S-tile (pair of ktiles):
               ScalarE: true Exp -> fp16
               DVE:     Schraudolph fp16 magic (one fp32 mult-add; low 2 bytes
                        of each fp32 = the fp16 weight, read at stride 2)
               GPSIMD:  same magic trick on a DMA-staged SBUF copy of the
                        scores (Pool has no PSUM port).
  AV       : q-major fp16-moving matmuls; per-qb sequential 16-step chains into
             one packed psum bank [128, 4, 65(pad 128)].
  norm     : DVE reciprocal + scalar_tensor_tensor broadcast-mult -> fp16 a2;
             aT via DMA xbar transpose (no PE, no psum).
  proj     : fp16 matmuls; ScalarE Identity applies 1/(SW*SP) + per-partition
             bias; yT [C, N] fp32 out (host transposes).
"""

import math

import numpy as np
import ml_dtypes

import concourse.bass as bass
import concourse.mybir as mybir
import concourse.tile as tile
from concourse import bacc
from concourse.bass_utils import run_bass_kernel_spmd

B, N, C, H = 8, 2048, 768, 12
D = C // H            # 64
CT = C // 128         # 6 channel tiles
NQ = 512              # query chunk (1 psum bank fp32)
NCH = N // NQ         # 4
NKT = N // 128        # 16 key tiles

SW = 32.0             # host scale on w_qkv
SP = 32.0             # host scale on w_proj
SCALE = float(D) ** -0.5
S_SCALE = SCALE / (SW * SW)        # exp scale on raw q8.k8 psum scores
LN2 = math.log(2.0)
A16 = 1024.0 * S_SCALE / LN2       # fp16 magic slope
B16 = 15301.5  # centered fp16 exponent bias (+.5 for trunc converts)

# Per-head-parity exp-lane assignment over the 8 S-tiles (pairs of ktiles).
# Act is the faster exp engine (0.83 vs 1.04 ns/row): give it the bigger share.
LANES = (
    "ADADADAA",   # head parity 0
    "DADAADAD",   # head parity 1
)
# evac lane for q/k f-major downcasts and v downcasts, by slice parity
QK_EVAC = ("D", "D")
V_EVAC = ("A", "A")

FP32 = mybir.dt.float32
FP16 = mybir.dt.float16
F8 = mybir.dt.float8e4
EXP = mybir.ActivationFunctionType.Exp
IDENT = mybir.ActivationFunctionType.Identity
COPY = mybir.ActivationFunctionType.Copy
MULT = mybir.AluOpType.mult
ADD = mybir.AluOpType.add
DR = mybir.MatmulPerfMode.DoubleRow

F8NP = ml_dtypes.float8_e4m3

_CACHED_NC = None


def _ap(base, free_dims):
    """AP with base's partition dim and explicit [stride, count] free dims."""
    return bass.AP(
        tensor=base.tensor,
        offset=base.offset,
        ap=[list(base.ap[0])] + [list(d) for d in free_dims],
    )


def build():
    nc = bacc.Bacc()
    x_hi = nc.dram_tensor("x_hi", [128, CT, N], F8, kind="ExternalInput")
    x_lo = nc.dram_tensor("x_lo", [128, CT, N], F8, kind="ExternalInput")
    w_hi = nc.dram_tensor("w_hi", [128, CT, 3 * C], F8, kind="ExternalInput")
    w_lo = nc.dram_tensor("w_lo", [128, CT, 3 * C], F8, kind="ExternalInput")
    wp = nc.dram_tensor("wp", [128, CT, C], FP16, kind="ExternalInput")
    b2d = nc.dram_tensor("b2d", [128, CT], FP32, kind="ExternalInput")
    yT = nc.dram_tensor("yT", [C, N], FP32, kind="ExternalOutput")

    lp = nc.allow_low_precision("fp8/fp16 matmuls with fp32 psum accumulation")
    lp.__enter__()
    with tile.TileContext(nc) as tc:
        with tc.tile_pool(name="big", bufs=1) as big, \
             tc.tile_pool(name="fmp", bufs=2) as fmp, \
             tc.tile_pool(name="e16p", bufs=7) as e16p, \
             tc.tile_pool(name="e32p", bufs=8) as e32p, \
             tc.tile_pool(name="small", bufs=2) as small, \
             tc.tile_pool(name="ps2p", bufs=3, space="PSUM") as ps2p, \
             tc.tile_pool(name="psavp", bufs=2, space="PSUM") as psavp:

            # ---- persistent inputs -------------------------------------
            # loaded per ct-pair so the first DR chains start early
            xh = big.tile([128, CT, N], F8)
            xl = big.tile([128, CT, N], F8)
            wh = big.tile([128, CT, 3 * C], F8)
            wl = big.tile([128, CT, 3 * C], F8)
            wh16 = big.tile([128, CT, 3 * C], F8)
            xh16 = big.tile([128, CT, N], F8)
            for t in range(3):
                ts = slice(2 * t, 2 * t + 2)
                nc.sync.dma_start(out=wh[:, ts, :], in_=w_hi[:, ts, :])
                nc.sync.dma_start(out=xh[:, ts, :], in_=x_hi[:, ts, :])
                nc.sync.dma_start(out=xl[:, ts, :], in_=x_lo[:, ts, :])
                nc.sync.dma_start(out=wl[:, ts, :], in_=w_lo[:, ts, :])
            # derive the 1/16-prescaled operands on-chip (engines idle here)
            for t in range(3):
                ts = slice(2 * t, 2 * t + 2)
                nc.scalar.activation(out=wh16[:, ts, :], in_=wh[:, ts, :],
                                     func=COPY, scale=1.0 / 16.0)
                nc.vector.tensor_scalar(out=xh16[:, ts, :], in0=xh[:, ts, :],
                                        scalar1=1.0 / 16.0, scalar2=None,
                                        op0=MULT)
            wpt = big.tile([128, CT, C], FP16)
            nc.sync.dma_start(out=wpt, in_=wp[:, :, :])
            bias = big.tile([128, CT], FP32)
            nc.sync.dma_start(out=bias, in_=b2d[:, :])

            qg = [big.tile([128, 2, N], F8, name=f"qg{g}") for g in range(4)]
            kg = [big.tile([128, 2, N], F8, name=f"kg{g}") for g in range(4)]
            vaug = [
                big.tile([128, NKT, 130], FP16, name=f"vaug{p}") for p in range(6)
            ]
            for p in range(6):
                nc.gpsimd.memset(vaug[p][:, :, 64:65], 1.0)
                nc.gpsimd.memset(vaug[p][:, :, 129:130], 1.0)
            aT = big.tile([128, CT, N], FP16)

            # residual-DR chain: 9 matmuls into one psum, full precision.
            # pairs: (hi, hi), (hi16, lo), (lo16, hi) on (weights, x) — caller
            # passes the already-matched (lhsT, rhs) AP pairs.
            def dr_chain(ps_out, pairs):
                k = 0
                for (lt, lsl), (rt, rsl) in pairs:
                    for t in range(3):
                        nc.tensor.matmul(
                            ps_out,
                            lt[:, 2 * t:2 * t + 2, lsl],
                            rt[:, 2 * t:2 * t + 2, rsl],
                            start=(k == 0), stop=(k == 8), perf_mode=DR,
                        )
                        k += 1

            def evac(lane, out, in_):
                if lane == "A":
                    nc.scalar.activation(out=out, in_=in_, func=COPY, scale=1.0)
                else:
                    nc.vector.tensor_copy(out, in_)

            # ---- phase A pieces ----------------------------------------
            def emit_qk_side(hp, ch, side, fms):
                n0 = ch * NQ
                nsl = slice(n0, n0 + NQ)
                ps = ps2p.tile([128, 2, NQ], FP32, tag="ps2", name="psqk")
                f0 = 128 * (hp + 6 * side)
                fsl = slice(f0, f0 + 128)
                dr_chain(
                    ps[:, 0, :],
                    (((wh, fsl), (xh, nsl)),
                     ((wh16, fsl), (xl, nsl)),
                     ((wl, fsl), (xh16, nsl))),
                )
                evac(QK_EVAC[(ch + side) % 2],
                     fms[side][:, n0:n0 + NQ], ps[:, 0, :])

            def emit_v_half(hp, ch, j):
                # 2 key-tiles of this pair's V: kts {4ch+2j, 4ch+2j+1}
                vf0 = 1536 + 128 * hp
                vsl = slice(vf0, vf0 + 128)
                ps = ps2p.tile([128, 2, NQ], FP32, tag="ps2", name="psv")
                for i in (0, 1):
                    kt = 4 * ch + 2 * j + i
                    n0 = kt * 128
                    nsl = slice(n0, n0 + 128)
                    dr_chain(
                        ps[:, i, 0:128],
                        (((xh, nsl), (wh, vsl)),
                         ((xl, nsl), (wh16, vsl)),
                         ((xh16, nsl), (wl, vsl))),
                    )
                vrow = vaug[hp][:, 4 * ch + 2 * j, :]
                out = _ap(vrow, [[130, 2], [65, 2], [1, 64]])
                src = _ap(ps[:, 0, :], [[512, 2], [64, 2], [1, 64]])
                evac(V_EVAC[(ch + j) % 2], out, src)

            def new_fm():
                fq = fmp.tile([128, N], F8, tag="fmq", name="fq")
                fk = fmp.tile([128, N], F8, tag="fmk", name="fk")
                return fq, fk

            def emit_a_slice(hp, ch, fms):
                emit_qk_side(hp, ch, 0, fms)
                emit_v_half(hp, ch, 0)
                emit_qk_side(hp, ch, 1, fms)
                emit_v_half(hp, ch, 1)

            def emit_rearrange_ch(hp, ch, fms):
                # fold [64, NQ] f-major chunk into [32, 2, NQ] (d = 2p + i)
                n0 = ch * NQ
                for side, grps in ((0, qg), (1, kg)):
                    for hi in (0, 1):
                        h = 2 * hp + hi
                        g, q4 = h // 3, h % 3
                        nc.sync.dma_start(
                            out=grps[g][32 * q4:32 * q4 + 32, :, n0:n0 + NQ],
                            in_=fms[side][64 * hi:64 * hi + 64, n0:n0 + NQ],
                        )

            # ---- phase B: S+exp for head k runs while head k-1's AV/norm
            # retires, so the exp engines always have fresh psums ------------
            def emit_s_exp(hp, ch, hi):
                n0 = ch * NQ
                h = 2 * hp + hi
                g, q4 = h // 3, h % 3
                p0 = 32 * q4
                qs = qg[g][p0:p0 + 32, :, n0:n0 + NQ]
                lanes = LANES[hi]
                ets = []
                for st in range(8):
                    ps = ps2p.tile([128, 2, NQ], FP32, tag="ps2", name="sps")
                    for tt in (0, 1):
                        kt = 2 * st + tt
                        nc.tensor.matmul(
                            ps[:, tt, :],
                            kg[g][p0:p0 + 32, :, kt * 128:kt * 128 + 128],
                            qs,
                            start=True, stop=True, perf_mode=DR,
                        )
                    if lanes[st] == "A":
                        et = e16p.tile([128, 2, NQ], FP16, tag="e16", name="e16")
                        nc.scalar.activation(
                            out=et, in_=ps, func=EXP, scale=S_SCALE
                        )
                        ets.append((et, False))
                    else:
                        # fp32 -> uint16 convert IS the magic: i16 lands as the
                        # fp16 bit pattern of ~exp(s)
                        et = e32p.tile([128, 2, NQ], mybir.dt.uint16,
                                       tag="e32", name="e32")
                        nc.vector.tensor_scalar(
                            out=et, in0=ps, scalar1=A16, scalar2=B16,
                            op0=MULT, op1=ADD,
                        )
                        ets.append((et, True))
                return (hp, ch, hi, ets)

            a2map = {}

            def emit_av_norm(ctx):
                hp, ch, hi, ets = ctx
                n0 = ch * NQ
                if hi == 0:
                    a2map[(hp, ch)] = small.tile(
                        [128, 4, 128], FP16, tag="a2", name="a2", bufs=2
                    )
                a2 = a2map[(hp, ch)]
                av = psavp.tile([128, 4, 128], FP32, tag="av", name="av")
                for qb in range(4):
                    for st in range(8):
                        et, magic = ets[st]
                        for tt in (0, 1):
                            kt = 2 * st + tt
                            if magic:
                                l = et.bitcast(FP16)[:, tt, qb * 128:qb * 128 + 128]
                            else:
                                l = et[:, tt, qb * 128:qb * 128 + 128]
                            nc.tensor.matmul(
                                av[:, qb, 0:65],
                                l,
                                vaug[hp][:, kt, 65 * hi:65 * hi + 65],
                                start=(st == 0 and tt == 0),
                                stop=(st == 7 and tt == 1),
                            )
                recp = small.tile([128, 4], FP32, tag="recp", name="recp")
                nc.vector.reciprocal(recp, av[:, :, 64])
                nc.vector.scalar_tensor_tensor(
                    out=a2[:, :, 64 * hi:64 * hi + 64],
                    in0=av[:, :, 0:64],
                    scalar=1.0,
                    in1=_ap(recp, [[1, 4], [0, 64]]),
                    op0=MULT, op1=MULT,
                )
                if hi == 1:
                    nc.sync.dma_start_transpose(
                        out=aT[:, hp, n0:n0 + NQ].rearrange(
                            "p (qb q) -> p qb q", qb=4
                        ),
                        in_=a2.rearrange("p qb d -> p (qb d)"),
                    )
                    del a2map[(hp, ch)]

            # ---- phase C ------------------------------------------------
            def emit_c():
                for ot in range(6):
                    for ch in range(NCH):
                        n0 = ch * NQ
                        ps = ps2p.tile([128, 2, NQ], FP32, tag="ps2", name="cps")
                        for ct in range(CT):
                            nc.tensor.matmul(
                                ps[:, 0, :],
                                wpt[:, ct, 128 * ot:128 * ot + 128],
                                aT[:, ct, n0:n0 + NQ],
                                start=(ct == 0), stop=(ct == CT - 1),
                            )
                        ysb = e32p.tile([128, NQ], FP32, tag="e32", name="ysb")
                        nc.scalar.activation(
                            out=ysb, in_=ps[:, 0, :], func=IDENT,
                            scale=1.0 / (SW * SP), bias=bias[:, ot:ot + 1],
                        )
                        nc.sync.dma_start(
                            out=yT[128 * ot:128 * ot + 128, n0:n0 + NQ], in_=ysb
                        )

            # ---- emission: A slices pipelined one pair ahead of B, AV
            # blocks deferred one head behind their S+exp ------------------
            fms = new_fm()
            for ch in range(NCH):
                emit_a_slice(0, ch, fms)
                emit_rearrange_ch(0, ch, fms)
            pend = None
            for hp in range(6):
                nfms = new_fm() if hp < 5 else None
                for ch in range(NCH):
                    for hi in (0, 1):
                        ctx = emit_s_exp(hp, ch, hi)
                        if pend is not None:
                            emit_av_norm(pend)
                        pend = ctx
                        if hp < 5:
                            emit_qk_side(hp + 1, ch, hi, nfms)
                            emit_v_half(hp + 1, ch, hi)
                            if hi == 1:
                                emit_rearrange_ch(hp + 1, ch, nfms)
            emit_av_norm(pend)
            emit_c()
    lp.__exit__(None, None, None)

    nc.finalize()
    return nc


def get_nc():
    global _CACHED_NC
    if _CACHED_NC is None:
        _CACHED_NC = build()
    return _CACHED_NC


def _prep_shared(w_qkv, w_proj, b_proj):
    wq = (w_qkv.astype(np.float64) * SW).astype(np.float32)
    w_hi = wq.astype(F8NP)
    w_lo = ((wq - w_hi.astype(np.float32)) * 16.0).astype(F8NP)

    def lay_w(a):
        return np.ascontiguousarray(a.T.reshape(CT, 128, 3 * C).transpose(1, 0, 2))
    wpm = (w_proj.astype(np.float64) * SP).astype(np.float16)
    wp_l = np.ascontiguousarray(wpm.T.reshape(CT, 128, C).transpose(1, 0, 2))
    b2d = np.ascontiguousarray(b_proj.reshape(CT, 128).T.astype(np.float32))
    return lay_w(w_hi), lay_w(w_lo), wp_l, b2d


def _prep_x(xi):
    xs = np.ascontiguousarray(xi.T.reshape(CT, 128, N).transpose(1, 0, 2))
    x_hi = xs.astype(F8NP)
    x_lo = ((xs - x_hi.astype(np.float32)) * 16.0).astype(F8NP)
    return x_hi, x_lo


LAST_RESULT = None


def kernel(x, w_qkv, w_proj, b_proj, **run_kwargs):
    x = np.ascontiguousarray(np.asarray(x, dtype=np.float32))
    w_qkv = np.ascontiguousarray(np.asarray(w_qkv, dtype=np.float32))
    w_proj = np.ascontiguousarray(np.asarray(w_proj, dtype=np.float32))
    b_proj = np.ascontiguousarray(np.asarray(b_proj, dtype=np.float32))
    assert x.shape == (B, N, C)

    nc = get_nc()
    w_hi, w_lo_l, wp_l, b2d = _prep_shared(w_qkv, w_proj, b_proj)
    in_maps = []
    for i in range(B):
        x_hi, x_lo = _prep_x(x[i])
        in_maps.append({
            "x_hi": x_hi, "x_lo": x_lo,
            "w_hi": w_hi, "w_lo": w_lo_l,
            "wp": wp_l, "b2d": b2d,
        })
    res = run_bass_kernel_spmd(nc, in_maps, list(range(B)), **run_kwargs)
    global LAST_RESULT
    LAST_RESULT = res
    out = np.stack(
        [np.ascontiguousarray(res.results[i]["yT"].T) for i in range(B)], axis=0
    )
    return out


if __name__ == "__main__":
    rng = np.random.default_rng(0)
    x = rng.standard_normal((B, N, C), dtype=np.float32)
    w_qkv = (rng.standard_normal((3 * C, C)) * 0.02).astype(np.float32)
    w_proj = (rng.standard_normal((C, C)) * 0.02).astype(np.float32)
    b_proj = (rng.standard_normal((C,)) * 0.02).astype(np.float32)
    out = kernel(x=x, w_qkv=w_qkv, w_proj=w_proj, b_proj=b_proj)
    print("out", out.shape, out.dtype, float(np.abs(out).max()))


# revision 6
# speedup vs baseline: 1.4954x; 1.0213x over previous
"""Multi-head attention Trainium2 Bass kernel, v7.

B=8, N=2048, C=768, H=12, D=64. Data-parallel over batch: 1 element/core.

Per-core pipeline:
  QKV q,k  : fp8e4 DoubleRow residual matmuls.  The 1/16 residual scale is
             pre-baked into host copies of the weights (w_hi/16, w_lo/16), so
             the 9 DR matmuls accumulate the full-precision result into ONE
             psum chain; the evacuation is a single-input downcast that can run
             on either ScalarE (activation Copy) or DVE (tensor_scalar).
  V        : same residual-DR trick, key-major -> fp16 vaug [128, kt, 130]
             (two heads' 64+1 slots; ones col = softmax denominator).
  S^T      : fp8 DoubleRow, D=64 split across the two DR slots (d = 2p+i on 32
             partitions); 0.5 cycles/row.  q8/k8 reach [32,2,N] via an
             SBUF->SBUF DMA fold.
  exp      : two lanes, assigned per S-tile (pair of ktiles; GPSIMD cannot
             help: no PSUM port, and DMA cannot read PSUM either):
               ScalarE: true Exp -> fp16
               DVE:     Schraudolph magic in ONE tensor_scalar: uint16 out =
                        round(s*1024*log2(e)*scale + 15301.5), whose bit
                        pattern read as fp16 is ~exp(s*scale).
  AV       : q-major fp16-moving matmuls; per-qb sequential 16-step chains into
             one packed psum bank [128, 4, 65(pad 128)].
  norm     : DVE reciprocal + scalar_tensor_tensor broadcast-mult -> fp16 a2;
             aT via DMA xbar transpose (no PE, no psum).
  proj     : fp16 matmuls; ScalarE Identity applies 1/(SW*SP) + per-partition
             bias; yT [C, N] fp32 out (host transposes).
"""

import math

import numpy as np
import ml_dtypes

import concourse.bass as bass
import concourse.mybir as mybir
import concourse.tile as tile
from concourse import bacc
from concourse.bass_utils import run_bass_kernel_spmd

B, N, C, H = 8, 2048, 768, 12
D = C // H            # 64
CT = C // 128         # 6 channel tiles
NQ = 512              # query chunk (1 psum bank fp32)
NCH = N // NQ         # 4
NKT = N // 128        # 16 key tiles

SW = 32.0             # host scale on w_qkv
SP = 32.0             # host scale on w_proj
SCALE = float(D) ** -0.5
S_SCALE = SCALE / (SW * SW)        # exp scale on raw q8.k8 psum scores
LN2 = math.log(2.0)
A16 = 1024.0 * S_SCALE / LN2       # fp16 magic slope
B16 = 15301.5  # centered fp16 exponent bias (+.5 for trunc converts)

# Per-head-parity exp-lane assignment over the 8 S-tiles (pairs of ktiles).
# Act is the faster exp engine (0.83 vs 1.04 ns/row): give it the bigger share.
LANES = (
    "ADADADAA",   # head parity 0
    "DADAADAD",   # head parity 1
)
# evac lane for q/k f-major downcasts and v downcasts, by slice parity
QK_EVAC = ("D", "D")
V_EVAC = ("A", "A")

FP32 = mybir.dt.float32
FP16 = mybir.dt.float16
F8 = mybir.dt.float8e4
EXP = mybir.ActivationFunctionType.Exp
IDENT = mybir.ActivationFunctionType.Identity
COPY = mybir.ActivationFunctionType.Copy
MULT = mybir.AluOpType.mult
ADD = mybir.AluOpType.add
DR = mybir.MatmulPerfMode.DoubleRow

F8NP = ml_dtypes.float8_e4m3

_CACHED_NC = None


def _ap(base, free_dims):
    """AP with base's partition dim and explicit [stride, count] free dims."""
    return bass.AP(
        tensor=base.tensor,
        offset=base.offset,
        ap=[list(base.ap[0])] + [list(d) for d in free_dims],
    )


def build():
    nc = bacc.Bacc()
    x_hi = nc.dram_tensor("x_hi", [128, CT, N], F8, kind="ExternalInput")
    x_lo = nc.dram_tensor("x_lo", [128, CT, N], F8, kind="ExternalInput")
    w_hi = nc.dram_tensor("w_hi", [128, CT, 3 * C], F8, kind="ExternalInput")
    w_lo = nc.dram_tensor("w_lo", [128, CT, 3 * C], F8, kind="ExternalInput")
    wp = nc.dram_tensor("wp", [128, CT, C], FP16, kind="ExternalInput")
    b2d = nc.dram_tensor("b2d", [128, CT], FP32, kind="ExternalInput")
    yT = nc.dram_tensor("yT", [C, N], FP32, kind="ExternalOutput")

    lp = nc.allow_low_precision("fp8/fp16 matmuls with fp32 psum accumulation")
    lp.__enter__()
    with tile.TileContext(nc) as tc:
        with tc.tile_pool(name="big", bufs=1) as big, \
             tc.tile_pool(name="fmp", bufs=2) as fmp, \
             tc.tile_pool(name="e16p", bufs=7) as e16p, \
             tc.tile_pool(name="e32p", bufs=8) as e32p, \
             tc.tile_pool(name="small", bufs=2) as small, \
             tc.tile_pool(name="ps2p", bufs=3, space="PSUM") as ps2p, \
             tc.tile_pool(name="psap", bufs=1, space="PSUM") as psap, \
             tc.tile_pool(name="psavp", bufs=1, space="PSUM") as psavp:

            # ---- persistent inputs -------------------------------------
            # loaded per ct-pair so the first DR chains start early
            xh = big.tile([128, CT, N], F8)
            xl = big.tile([128, CT, N], F8)
            wh = big.tile([128, CT, 3 * C], F8)
            wl = big.tile([128, CT, 3 * C], F8)
            wh16 = big.tile([128, CT, 3 * C], F8)
            xh16 = big.tile([128, CT, N], F8)
            for t in range(3):
                ts = slice(2 * t, 2 * t + 2)
                nc.sync.dma_start(out=wh[:, ts, :], in_=w_hi[:, ts, :])
                nc.sync.dma_start(out=xh[:, ts, :], in_=x_hi[:, ts, :])
                nc.sync.dma_start(out=xl[:, ts, :], in_=x_lo[:, ts, :])
                nc.sync.dma_start(out=wl[:, ts, :], in_=w_lo[:, ts, :])
            # derive the 1/16-prescaled operands on-chip (engines idle here)
            for t in range(3):
                ts = slice(2 * t, 2 * t + 2)
                nc.scalar.activation(out=wh16[:, ts, :], in_=wh[:, ts, :],
                                     func=COPY, scale=1.0 / 16.0)
                nc.vector.tensor_scalar(out=xh16[:, ts, :], in0=xh[:, ts, :],
                                        scalar1=1.0 / 16.0, scalar2=None,
                                        op0=MULT)
            wpt = big.tile([128, CT, C], FP16)
            nc.sync.dma_start(out=wpt, in_=wp[:, :, :])
            bias = big.tile([128, CT], FP32)
            nc.sync.dma_start(out=bias, in_=b2d[:, :])

            qg = [big.tile([128, 2, N], F8, name=f"qg{g}") for g in range(4)]
            kg = [big.tile([128, 2, N], F8, name=f"kg{g}") for g in range(4)]
            vaug = [
                big.tile([128, NKT, 130], FP16, name=f"vaug{p}") for p in range(6)
            ]
            for p in range(6):
                nc.gpsimd.memset(vaug[p][:, :, 64:65], 1.0)
                nc.gpsimd.memset(vaug[p][:, :, 129:130], 1.0)
            aT = big.tile([128, CT, N], FP16)

            # residual-DR chain: 9 matmuls into one psum, full precision.
            # pairs: (hi, hi), (hi16, lo), (lo16, hi) on (weights, x) — caller
            # passes the already-matched (lhsT, rhs) AP pairs.
            def dr_chain(ps_out, pairs):
                k = 0
                for (lt, lsl), (rt, rsl) in pairs:
                    for t in range(3):
                        nc.tensor.matmul(
                            ps_out,
                            lt[:, 2 * t:2 * t + 2, lsl],
                            rt[:, 2 * t:2 * t + 2, rsl],
                            start=(k == 0), stop=(k == 8), perf_mode=DR,
                        )
                        k += 1

            def evac(lane, out, in_):
                if lane == "A":
                    nc.scalar.activation(out=out, in_=in_, func=COPY, scale=1.0)
                else:
                    nc.vector.tensor_copy(out, in_)

            # ---- phase A pieces ----------------------------------------
            def emit_qk_side(hp, ch, side, fms):
                n0 = ch * NQ
                nsl = slice(n0, n0 + NQ)
                ps = psap.tile([128, NQ], FP32, tag="psa", name="psqk")
                f0 = 128 * (hp + 6 * side)
                fsl = slice(f0, f0 + 128)
                dr_chain(
                    ps[:, :],
                    (((wh, fsl), (xh, nsl)),
                     ((wh16, fsl), (xl, nsl)),
                     ((wl, fsl), (xh16, nsl))),
                )
                evac(QK_EVAC[(ch + side) % 2],
                     fms[side][:, n0:n0 + NQ], ps[:, :])

            def emit_v_half(hp, ch, j):
                # 2 key-tiles of this pair's V: kts {4ch+2j, 4ch+2j+1}
                vf0 = 1536 + 128 * hp
                vsl = slice(vf0, vf0 + 128)
                ps = psap.tile([128, NQ], FP32, tag="psa", name="psv")
                for i in (0, 1):
                    kt = 4 * ch + 2 * j + i
                    n0 = kt * 128
                    nsl = slice(n0, n0 + 128)
                    dr_chain(
                        ps[:, 128 * i:128 * i + 128],
                        (((xh, nsl), (wh, vsl)),
                         ((xl, nsl), (wh16, vsl)),
                         ((xh16, nsl), (wl, vsl))),
                    )
                vrow = vaug[hp][:, 4 * ch + 2 * j, :]
                out = _ap(vrow, [[130, 2], [65, 2], [1, 64]])
                src = _ap(ps[:, :], [[128, 2], [64, 2], [1, 64]])
                evac(V_EVAC[(ch + j) % 2], out, src)

            def new_fm():
                fq = fmp.tile([128, N], F8, tag="fmq", name="fq")
                fk = fmp.tile([128, N], F8, tag="fmk", name="fk")
                return fq, fk

            def emit_a_slice(hp, ch, fms):
                emit_qk_side(hp, ch, 0, fms)
                emit_v_half(hp, ch, 0)
                emit_qk_side(hp, ch, 1, fms)
                emit_v_half(hp, ch, 1)

            def emit_rearrange_ch(hp, ch, fms):
                # fold [64, NQ] f-major chunk into [32, 2, NQ] (d = 2p + i)
                n0 = ch * NQ
                for side, grps in ((0, qg), (1, kg)):
                    for hi in (0, 1):
                        h = 2 * hp + hi
                        g, q4 = h // 3, h % 3
                        nc.sync.dma_start(
                            out=grps[g][32 * q4:32 * q4 + 32, :, n0:n0 + NQ],
                            in_=fms[side][64 * hi:64 * hi + 64, n0:n0 + NQ],
                        )

            # ---- phase B: S+exp for head k runs while head k-1's AV/norm
            # retires, so the exp engines always have fresh psums ------------
            def emit_s_exp(hp, ch, hi):
                n0 = ch * NQ
                h = 2 * hp + hi
                g, q4 = h // 3, h % 3
                p0 = 32 * q4
                qs = qg[g][p0:p0 + 32, :, n0:n0 + NQ]
                lanes = LANES[hi]
                ets = []
                for st in range(8):
                    ps = ps2p.tile([128, 2, NQ], FP32, tag="ps2", name="sps")
                    for tt in (0, 1):
                        kt = 2 * st + tt
                        nc.tensor.matmul(
                            ps[:, tt, :],
                            kg[g][p0:p0 + 32, :, kt * 128:kt * 128 + 128],
                            qs,
                            start=True, stop=True, perf_mode=DR,
                        )
                    if lanes[st] == "A":
                        et = e16p.tile([128, 2, NQ], FP16, tag="e16", name="e16")
                        nc.scalar.activation(
                            out=et, in_=ps, func=EXP, scale=S_SCALE
                        )
                        ets.append((et, False))
                    else:
                        # fp32 -> uint16 convert IS the magic: i16 lands as the
                        # fp16 bit pattern of ~exp(s)
                        et = e32p.tile([128, 2, NQ], mybir.dt.uint16,
                                       tag="e32", name="e32")
                        nc.vector.tensor_scalar(
                            out=et, in0=ps, scalar1=A16, scalar2=B16,
                            op0=MULT, op1=ADD,
                        )
                        ets.append((et, True))
                return (hp, ch, hi, ets)

            a2map = {}

            def emit_av_norm(ctx):
                hp, ch, hi, ets = ctx
                n0 = ch * NQ
                if hi == 0:
                    a2map[(hp, ch)] = small.tile(
                        [128, 4, 128], FP16, tag="a2", name="a2", bufs=2
                    )
                a2 = a2map[(hp, ch)]
                av = psavp.tile([128, 4, 128], FP32, tag="av", name="av")
                for qb in range(4):
                    for st in range(8):
                        et, magic = ets[st]
                        for tt in (0, 1):
                            kt = 2 * st + tt
                            if magic:
                                l = et.bitcast(FP16)[:, tt, qb * 128:qb * 128 + 128]
                            else:
                                l = et[:, tt, qb * 128:qb * 128 + 128]
                            nc.tensor.matmul(
                                av[:, qb, 0:65],
                                l,
                                vaug[hp][:, kt, 65 * hi:65 * hi + 65],
                                start=(st == 0 and tt == 0),
                                stop=(st == 7 and tt == 1),
                            )
                recp = small.tile([128, 4], FP32, tag="recp", name="recp")
                nc.vector.reciprocal(recp, av[:, :, 64])
                nc.vector.scalar_tensor_tensor(
                    out=a2[:, :, 64 * hi:64 * hi + 64],
                    in0=av[:, :, 0:64],
                    scalar=1.0,
                    in1=_ap(recp, [[1, 4], [0, 64]]),
                    op0=MULT, op1=MULT,
                )
                if hi == 1:
                    nc.sync.dma_start_transpose(
                        out=aT[:, hp, n0:n0 + NQ].rearrange(
                            "p (qb q) -> p qb q", qb=4
                        ),
                        in_=a2.rearrange("p qb d -> p (qb d)"),
                    )
                    del a2map[(hp, ch)]

            # ---- phase C ------------------------------------------------
            def emit_c():
                for ot in range(6):
                    for ch in range(NCH):
                        n0 = ch * NQ
                        ps = ps2p.tile([128, 2, NQ], FP32, tag="ps2", name="cps")
                        for ct in range(CT):
                            nc.tensor.matmul(
                                ps[:, 0, :],
                                wpt[:, ct, 128 * ot:128 * ot + 128],
                                aT[:, ct, n0:n0 + NQ],
                                start=(ct == 0), stop=(ct == CT - 1),
                            )
                        ysb = e32p.tile([128, NQ], FP32, tag="e32", name="ysb")
                        nc.scalar.activation(
                            out=ysb, in_=ps[:, 0, :], func=IDENT,
                            scale=1.0 / (SW * SP), bias=bias[:, ot:ot + 1],
                        )
                        nc.sync.dma_start(
                            out=yT[128 * ot:128 * ot + 128, n0:n0 + NQ], in_=ysb
                        )

            # ---- emission: A slices pipelined one pair ahead of B, AV
            # blocks deferred one head behind their S+exp ------------------
            fms = new_fm()
            for ch in range(NCH):
                emit_a_slice(0, ch, fms)
                emit_rearrange_ch(0, ch, fms)
            pend = None
            for hp in range(6):
                nfms = new_fm() if hp < 5 else None
                for ch in range(NCH):
                    for hi in (0, 1):
                        ctx = emit_s_exp(hp, ch, hi)
                        if pend is not None:
                            emit_av_norm(pend)
                        pend = ctx
                        if hp < 5:
                            emit_qk_side(hp + 1, ch, hi, nfms)
                            emit_v_half(hp + 1, ch, hi)
                            if hi == 1:
                                emit_rearrange_ch(hp + 1, ch, nfms)
            emit_av_norm(pend)
            emit_c()
    lp.__exit__(None, None, None)

    nc.finalize()
    return nc


def get_nc():
    global _CACHED_NC
    if _CACHED_NC is None:
        _CACHED_NC = build()
    return _CACHED_NC


def _prep_shared(w_qkv, w_proj, b_proj):
    wq = (w_qkv.astype(np.float64) * SW).astype(np.float32)
    w_hi = wq.astype(F8NP)
    w_lo = ((wq - w_hi.astype(np.float32)) * 16.0).astype(F8NP)

    def lay_w(a):
        return np.ascontiguousarray(a.T.reshape(CT, 128, 3 * C).transpose(1, 0, 2))
    wpm = (w_proj.astype(np.float64) * SP).astype(np.float16)
    wp_l = np.ascontiguousarray(wpm.T.reshape(CT, 128, C).transpose(1, 0, 2))
    b2d = np.ascontiguousarray(b_proj.reshape(CT, 128).T.astype(np.float32))
    return lay_w(w_hi), lay_w(w_lo), wp_l, b2d


def _prep_x(xi):
    xs = np.ascontiguousarray(xi.T.reshape(CT, 128, N).transpose(1, 0, 2))
    x_hi = xs.astype(F8NP)
    x_lo = ((xs - x_hi.astype(np.float32)) * 16.0).astype(F8NP)
    return x_hi, x_lo


LAST_RESULT = None


def kernel(x, w_qkv, w_proj, b_proj, **run_kwargs):
    x = np.ascontiguousarray(np.asarray(x, dtype=np.float32))
    w_qkv = np.ascontiguousarray(np.asarray(w_qkv, dtype=np.float32))
    w_proj = np.ascontiguousarray(np.asarray(w_proj, dtype=np.float32))
    b_proj = np.ascontiguousarray(np.asarray(b_proj, dtype=np.float32))
    assert x.shape == (B, N, C)

    nc = get_nc()
    w_hi, w_lo_l, wp_l, b2d = _prep_shared(w_qkv, w_proj, b_proj)
    in_maps = []
    for i in range(B):
        x_hi, x_lo = _prep_x(x[i])
        in_maps.append({
            "x_hi": x_hi, "x_lo": x_lo,
            "w_hi": w_hi, "w_lo": w_lo_l,
            "wp": wp_l, "b2d": b2d,
        })
    res = run_bass_kernel_spmd(nc, in_maps, list(range(B)), **run_kwargs)
    global LAST_RESULT
    LAST_RESULT = res
    out = np.stack(
        [np.ascontiguousarray(res.results[i]["yT"].T) for i in range(B)], axis=0
    )
    return out


if __name__ == "__main__":
    rng = np.random.default_rng(0)
    x = rng.standard_normal((B, N, C), dtype=np.float32)
    w_qkv = (rng.standard_normal((3 * C, C)) * 0.02).astype(np.float32)
    w_proj = (rng.standard_normal((C, C)) * 0.02).astype(np.float32)
    b_proj = (rng.standard_normal((C,)) * 0.02).astype(np.float32)
    out = kernel(x=x, w_qkv=w_qkv, w_proj=w_proj, b_proj=b_proj)
    print("out", out.shape, out.dtype, float(np.abs(out).max()))


# revision 7
# speedup vs baseline: 1.5285x; 1.0221x over previous
"""Multi-head attention Trainium2 Bass kernel, v7.

B=8, N=2048, C=768, H=12, D=64. Data-parallel over batch: 1 element/core.

Per-core pipeline:
  QKV q,k  : fp8e4 DoubleRow residual matmuls.  The 1/16 residual scale is
             pre-baked into host copies of the weights (w_hi/16, w_lo/16), so
             the 9 DR matmuls accumulate the full-precision result into ONE
             psum chain; the evacuation is a single-input downcast that can run
             on either ScalarE (activation Copy) or DVE (tensor_scalar).
  V        : same residual-DR trick, key-major -> fp16 vaug [128, kt, 130]
             (two heads' 64+1 slots; ones col = softmax denominator).
  S^T      : fp8 DoubleRow, D=64 split across the two DR slots (d = 2p+i on 32
             partitions); 0.5 cycles/row.  q8/k8 reach [32,2,N] via an
             SBUF->SBUF DMA fold.
  exp      : two lanes, assigned per S-tile (pair of ktiles; GPSIMD cannot
             help: no PSUM port, and DMA cannot read PSUM either):
               ScalarE: true Exp -> fp16
               DVE:     Schraudolph magic in ONE tensor_scalar: uint16 out =
                        round(s*1024*log2(e)*scale + 15301.5), whose bit
                        pattern read as fp16 is ~exp(s*scale).
  AV       : q-major fp16-moving matmuls; per-qb sequential 16-step chains into
             one packed psum bank [128, 4, 65(pad 128)].
  norm     : DVE reciprocal + scalar_tensor_tensor broadcast-mult -> fp16 a2;
             aT via DMA xbar transpose (no PE, no psum).
  proj     : fp16 matmuls; ScalarE Identity applies 1/(SW*SP) + per-partition
             bias; yT [C, N] fp32 out (host transposes).
"""

import math

import numpy as np
import ml_dtypes

import concourse.bass as bass
import concourse.mybir as mybir
import concourse.tile as tile
from concourse import bacc
from concourse.bass_utils import run_bass_kernel_spmd

B, N, C, H = 8, 2048, 768, 12
D = C // H            # 64
CT = C // 128         # 6 channel tiles
NQ = 512              # query chunk (1 psum bank fp32)
NCH = N // NQ         # 4
NKT = N // 128        # 16 key tiles

SW = 32.0             # host scale on w_qkv
SP = 32.0             # host scale on w_proj
SCALE = float(D) ** -0.5
S_SCALE = SCALE / (SW * SW)        # exp scale on raw q8.k8 psum scores
LN2 = math.log(2.0)
A16 = 1024.0 * S_SCALE / LN2       # fp16 magic slope
B16 = 15301.5  # centered fp16 exponent bias (+.5 for trunc converts)

# Per-head-parity exp-lane assignment over the 8 S-tiles (pairs of ktiles).
# Act is the faster exp engine (0.83 vs 1.04 ns/row): give it the bigger share.
LANES = (
    "ADADADAA",   # head parity 0
    "DADADAAD",   # head parity 1
)
# evac lane for q/k f-major downcasts and v downcasts, by slice parity
QK_EVAC = ("A", "D")
V_EVAC = ("D", "A")

FP32 = mybir.dt.float32
FP16 = mybir.dt.float16
F8 = mybir.dt.float8e4
EXP = mybir.ActivationFunctionType.Exp
IDENT = mybir.ActivationFunctionType.Identity
COPY = mybir.ActivationFunctionType.Copy
MULT = mybir.AluOpType.mult
ADD = mybir.AluOpType.add
DR = mybir.MatmulPerfMode.DoubleRow

F8NP = ml_dtypes.float8_e4m3

_CACHED_NC = None


def _ap(base, free_dims):
    """AP with base's partition dim and explicit [stride, count] free dims."""
    return bass.AP(
        tensor=base.tensor,
        offset=base.offset,
        ap=[list(base.ap[0])] + [list(d) for d in free_dims],
    )


def build():
    nc = bacc.Bacc()
    x_hi = nc.dram_tensor("x_hi", [128, CT, N], F8, kind="ExternalInput")
    x_lo = nc.dram_tensor("x_lo", [128, CT, N], F8, kind="ExternalInput")
    w_hi = nc.dram_tensor("w_hi", [128, CT, 3 * C], F8, kind="ExternalInput")
    w_lo = nc.dram_tensor("w_lo", [128, CT, 3 * C], F8, kind="ExternalInput")
    wp = nc.dram_tensor("wp", [128, CT, C], FP16, kind="ExternalInput")
    b2d = nc.dram_tensor("b2d", [128, CT], FP32, kind="ExternalInput")
    yT = nc.dram_tensor("yT", [C, N], FP32, kind="ExternalOutput")

    lp = nc.allow_low_precision("fp8/fp16 matmuls with fp32 psum accumulation")
    lp.__enter__()
    with tile.TileContext(nc) as tc:
        with tc.tile_pool(name="big", bufs=1) as big, \
             tc.tile_pool(name="fmp", bufs=2) as fmp, \
             tc.tile_pool(name="e16p", bufs=7) as e16p, \
             tc.tile_pool(name="e32p", bufs=8) as e32p, \
             tc.tile_pool(name="small", bufs=2) as small, \
             tc.tile_pool(name="ps2p", bufs=3, space="PSUM") as ps2p, \
             tc.tile_pool(name="psap", bufs=1, space="PSUM") as psap, \
             tc.tile_pool(name="psavp", bufs=1, space="PSUM") as psavp:

            # ---- persistent inputs -------------------------------------
            # loaded per ct-pair so the first DR chains start early
            xh = big.tile([128, CT, N], F8)
            xl = big.tile([128, CT, N], F8)
            wh = big.tile([128, CT, 3 * C], F8)
            wl = big.tile([128, CT, 3 * C], F8)
            wh16 = big.tile([128, CT, 3 * C], F8)
            xh16 = big.tile([128, CT, N], F8)
            for t in range(3):
                ts = slice(2 * t, 2 * t + 2)
                nc.sync.dma_start(out=wh[:, ts, :], in_=w_hi[:, ts, :])
                nc.sync.dma_start(out=xh[:, ts, :], in_=x_hi[:, ts, :])
                nc.sync.dma_start(out=xl[:, ts, :], in_=x_lo[:, ts, :])
                nc.sync.dma_start(out=wl[:, ts, :], in_=w_lo[:, ts, :])
            # derive the 1/16-prescaled operands on-chip (engines idle here)
            for t in range(3):
                ts = slice(2 * t, 2 * t + 2)
                nc.scalar.activation(out=wh16[:, ts, :], in_=wh[:, ts, :],
                                     func=COPY, scale=1.0 / 16.0)
                nc.vector.tensor_scalar(out=xh16[:, ts, :], in0=xh[:, ts, :],
                                        scalar1=1.0 / 16.0, scalar2=None,
                                        op0=MULT)
            wpt = big.tile([128, CT, C], FP16)
            nc.sync.dma_start(out=wpt, in_=wp[:, :, :])
            bias = big.tile([128, CT], FP32)
            nc.sync.dma_start(out=bias, in_=b2d[:, :])

            qg = [big.tile([128, 2, N], F8, name=f"qg{g}") for g in range(4)]
            kg = [big.tile([128, 2, N], F8, name=f"kg{g}") for g in range(4)]
            vaug = [
                big.tile([128, NKT, 130], FP16, name=f"vaug{p}") for p in range(6)
            ]
            for p in range(6):
                nc.gpsimd.memset(vaug[p][:, :, 64:65], 1.0)
                nc.gpsimd.memset(vaug[p][:, :, 129:130], 1.0)
            aT = big.tile([128, CT, N], FP16)

            # residual-DR chain: 9 matmuls into one psum, full precision.
            # pairs: (hi, hi), (hi16, lo), (lo16, hi) on (weights, x) — caller
            # passes the already-matched (lhsT, rhs) AP pairs.
            def dr_chain(ps_out, pairs):
                k = 0
                for (lt, lsl), (rt, rsl) in pairs:
                    for t in range(3):
                        nc.tensor.matmul(
                            ps_out,
                            lt[:, 2 * t:2 * t + 2, lsl],
                            rt[:, 2 * t:2 * t + 2, rsl],
                            start=(k == 0), stop=(k == 8), perf_mode=DR,
                        )
                        k += 1

            def evac(lane, out, in_):
                if lane == "A":
                    nc.scalar.activation(out=out, in_=in_, func=COPY, scale=1.0)
                else:
                    nc.vector.tensor_copy(out, in_)

            # ---- phase A pieces ----------------------------------------
            def emit_qk_side(hp, ch, side, fms):
                n0 = ch * NQ
                nsl = slice(n0, n0 + NQ)
                ps = psap.tile([128, NQ], FP32, tag="psa", name="psqk")
                f0 = 128 * (hp + 6 * side)
                fsl = slice(f0, f0 + 128)
                dr_chain(
                    ps[:, :],
                    (((wh, fsl), (xh, nsl)),
                     ((wh16, fsl), (xl, nsl)),
                     ((wl, fsl), (xh16, nsl))),
                )
                evac(QK_EVAC[(ch + side) % 2],
                     fms[side][:, n0:n0 + NQ], ps[:, :])

            def emit_v_half(hp, ch, j):
                # 2 key-tiles of this pair's V: kts {4ch+2j, 4ch+2j+1}
                vf0 = 1536 + 128 * hp
                vsl = slice(vf0, vf0 + 128)
                ps = psap.tile([128, NQ], FP32, tag="psa", name="psv")
                for i in (0, 1):
                    kt = 4 * ch + 2 * j + i
                    n0 = kt * 128
                    nsl = slice(n0, n0 + 128)
                    dr_chain(
                        ps[:, 128 * i:128 * i + 128],
                        (((xh, nsl), (wh, vsl)),
                         ((xl, nsl), (wh16, vsl)),
                         ((xh16, nsl), (wl, vsl))),
                    )
                vrow = vaug[hp][:, 4 * ch + 2 * j, :]
                out = _ap(vrow, [[130, 2], [65, 2], [1, 64]])
                src = _ap(ps[:, :], [[128, 2], [64, 2], [1, 64]])
                evac(V_EVAC[(ch + j) % 2], out, src)

            def new_fm():
                fq = fmp.tile([128, N], F8, tag="fmq", name="fq")
                fk = fmp.tile([128, N], F8, tag="fmk", name="fk")
                return fq, fk

            def emit_a_slice(hp, ch, fms):
                emit_qk_side(hp, ch, 0, fms)
                emit_v_half(hp, ch, 0)
                emit_qk_side(hp, ch, 1, fms)
                emit_v_half(hp, ch, 1)

            def emit_rearrange_ch(hp, ch, fms):
                # fold [64, NQ] f-major chunk into [32, 2, NQ] (d = 2p + i)
                n0 = ch * NQ
                for side, grps in ((0, qg), (1, kg)):
                    for hi in (0, 1):
                        h = 2 * hp + hi
                        g, q4 = h // 3, h % 3
                        nc.sync.dma_start(
                            out=grps[g][32 * q4:32 * q4 + 32, :, n0:n0 + NQ],
                            in_=fms[side][64 * hi:64 * hi + 64, n0:n0 + NQ],
                        )

            # ---- phase B: S+exp for head k runs while head k-1's AV/norm
            # retires, so the exp engines always have fresh psums ------------
            def emit_s_exp(hp, ch, hi):
                n0 = ch * NQ
                h = 2 * hp + hi
                g, q4 = h // 3, h % 3
                p0 = 32 * q4
                qs = qg[g][p0:p0 + 32, :, n0:n0 + NQ]
                lanes = LANES[hi]
                ets = []
                for st in range(8):
                    ps = ps2p.tile([128, 2, NQ], FP32, tag="ps2", name="sps")
                    for tt in (0, 1):
                        kt = 2 * st + tt
                        nc.tensor.matmul(
                            ps[:, tt, :],
                            kg[g][p0:p0 + 32, :, kt * 128:kt * 128 + 128],
                            qs,
                            start=True, stop=True, perf_mode=DR,
                        )
                    if lanes[st] == "A":
                        et = e16p.tile([128, 2, NQ], FP16, tag="e16", name="e16")
                        nc.scalar.activation(
                            out=et, in_=ps, func=EXP, scale=S_SCALE
                        )
                        ets.append((et, False))
                    else:
                        # fp32 -> uint16 convert IS the magic: i16 lands as the
                        # fp16 bit pattern of ~exp(s)
                        et = e32p.tile([128, 2, NQ], mybir.dt.uint16,
                                       tag="e32", name="e32")
                        nc.vector.tensor_scalar(
                            out=et, in0=ps, scalar1=A16, scalar2=B16,
                            op0=MULT, op1=ADD,
                        )
                        ets.append((et, True))
                return (hp, ch, hi, ets)

            a2map = {}

            def emit_av_norm(ctx):
                hp, ch, hi, ets = ctx
                n0 = ch * NQ
                if hi == 0:
                    a2map[(hp, ch)] = small.tile(
                        [128, 4, 128], FP16, tag="a2", name="a2", bufs=2
                    )
                a2 = a2map[(hp, ch)]
                av = psavp.tile([128, 4, 128], FP32, tag="av", name="av")
                for qb in range(4):
                    for st in range(8):
                        et, magic = ets[st]
                        for tt in (0, 1):
                            kt = 2 * st + tt
                            if magic:
                                l = et.bitcast(FP16)[:, tt, qb * 128:qb * 128 + 128]
                            else:
                                l = et[:, tt, qb * 128:qb * 128 + 128]
                            nc.tensor.matmul(
                                av[:, qb, 0:65],
                                l,
                                vaug[hp][:, kt, 65 * hi:65 * hi + 65],
                                start=(st == 0 and tt == 0),
                                stop=(st == 7 and tt == 1),
                            )
                recp = small.tile([128, 4], FP32, tag="recp", name="recp")
                nc.vector.reciprocal(recp, av[:, :, 64])
                nc.vector.scalar_tensor_tensor(
                    out=a2[:, :, 64 * hi:64 * hi + 64],
                    in0=av[:, :, 0:64],
                    scalar=1.0,
                    in1=_ap(recp, [[1, 4], [0, 64]]),
                    op0=MULT, op1=MULT,
                )
                if hi == 1:
                    nc.sync.dma_start_transpose(
                        out=aT[:, hp, n0:n0 + NQ].rearrange(
                            "p (qb q) -> p qb q", qb=4
                        ),
                        in_=a2.rearrange("p qb d -> p (qb d)"),
                    )
                    del a2map[(hp, ch)]

            # ---- phase C ------------------------------------------------
            def emit_c():
                for ot in range(6):
                    for ch in range(NCH):
                        n0 = ch * NQ
                        ps = ps2p.tile([128, 2, NQ], FP32, tag="ps2", name="cps")
                        for ct in range(CT):
                            nc.tensor.matmul(
                                ps[:, 0, :],
                                wpt[:, ct, 128 * ot:128 * ot + 128],
                                aT[:, ct, n0:n0 + NQ],
                                start=(ct == 0), stop=(ct == CT - 1),
                            )
                        ysb = e32p.tile([128, NQ], FP32, tag="e32", name="ysb")
                        nc.scalar.activation(
                            out=ysb, in_=ps[:, 0, :], func=IDENT,
                            scale=1.0 / (SW * SP), bias=bias[:, ot:ot + 1],
                        )
                        nc.sync.dma_start(
                            out=yT[128 * ot:128 * ot + 128, n0:n0 + NQ], in_=ysb
                        )

            # ---- emission: A slices pipelined one pair ahead of B, AV
            # blocks deferred one head behind their S+exp ------------------
            fms = new_fm()
            for ch in range(NCH):
                emit_a_slice(0, ch, fms)
                emit_rearrange_ch(0, ch, fms)
            pend = None
            for hp in range(6):
                nfms = new_fm() if hp < 5 else None
                for ch in range(NCH):
                    for hi in (0, 1):
                        ctx = emit_s_exp(hp, ch, hi)
                        if pend is not None:
                            emit_av_norm(pend)
                        pend = ctx
                        if hp < 5:
                            emit_qk_side(hp + 1, ch, hi, nfms)
                            emit_v_half(hp + 1, ch, hi)
                            if hi == 1:
                                emit_rearrange_ch(hp + 1, ch, nfms)
            emit_av_norm(pend)
            emit_c()
    lp.__exit__(None, None, None)

    nc.finalize()
    return nc


def get_nc():
    global _CACHED_NC
    if _CACHED_NC is None:
        _CACHED_NC = build()
    return _CACHED_NC


def _prep_shared(w_qkv, w_proj, b_proj):
    wq = (w_qkv.astype(np.float64) * SW).astype(np.float32)
    w_hi = wq.astype(F8NP)
    w_lo = ((wq - w_hi.astype(np.float32)) * 16.0).astype(F8NP)

    def lay_w(a):
        return np.ascontiguousarray(a.T.reshape(CT, 128, 3 * C).transpose(1, 0, 2))
    wpm = (w_proj.astype(np.float64) * SP).astype(np.float16)
    wp_l = np.ascontiguousarray(wpm.T.reshape(CT, 128, C).transpose(1, 0, 2))
    b2d = np.ascontiguousarray(b_proj.reshape(CT, 128).T.astype(np.float32))
    return lay_w(w_hi), lay_w(w_lo), wp_l, b2d


def _prep_x(xi):
    xs = np.ascontiguousarray(xi.T.reshape(CT, 128, N).transpose(1, 0, 2))
    x_hi = xs.astype(F8NP)
    x_lo = ((xs - x_hi.astype(np.float32)) * 16.0).astype(F8NP)
    return x_hi, x_lo


LAST_RESULT = None


def kernel(x, w_qkv, w_proj, b_proj, **run_kwargs):
    x = np.ascontiguousarray(np.asarray(x, dtype=np.float32))
    w_qkv = np.ascontiguousarray(np.asarray(w_qkv, dtype=np.float32))
    w_proj = np.ascontiguousarray(np.asarray(w_proj, dtype=np.float32))
    b_proj = np.ascontiguousarray(np.asarray(b_proj, dtype=np.float32))
    assert x.shape == (B, N, C)

    nc = get_nc()
    w_hi, w_lo_l, wp_l, b2d = _prep_shared(w_qkv, w_proj, b_proj)
    in_maps = []
    for i in range(B):
        x_hi, x_lo = _prep_x(x[i])
        in_maps.append({
            "x_hi": x_hi, "x_lo": x_lo,
            "w_hi": w_hi, "w_lo": w_lo_l,
            "wp": wp_l, "b2d": b2d,
        })
    res = run_bass_kernel_spmd(nc, in_maps, list(range(B)), **run_kwargs)
    global LAST_RESULT
    LAST_RESULT = res
    out = np.stack(
        [np.ascontiguousarray(res.results[i]["yT"].T) for i in range(B)], axis=0
    )
    return out


if __name__ == "__main__":
    rng = np.random.default_rng(0)
    x = rng.standard_normal((B, N, C), dtype=np.float32)
    w_qkv = (rng.standard_normal((3 * C, C)) * 0.02).astype(np.float32)
    w_proj = (rng.standard_normal((C, C)) * 0.02).astype(np.float32)
    b_proj = (rng.standard_normal((C,)) * 0.02).astype(np.float32)
    out = kernel(x=x, w_qkv=w_qkv, w_proj=w_proj, b_proj=b_proj)
    print("out", out.shape, out.dtype, float(np.abs(out).max()))


# revision 8
# speedup vs baseline: 1.5310x; 1.0016x over previous
"""Multi-head attention Trainium2 Bass kernel, v7.

B=8, N=2048, C=768, H=12, D=64. Data-parallel over batch: 1 element/core.

Per-core pipeline:
  QKV q,k  : fp8e4 DoubleRow residual matmuls.  The 1/16 residual scale is
             pre-baked into host copies of the weights (w_hi/16, w_lo/16), so
             the 9 DR matmuls accumulate the full-precision result into ONE
             psum chain; the evacuation is a single-input downcast that can run
             on either ScalarE (activation Copy) or DVE (tensor_scalar).
  V        : same residual-DR trick, key-major -> fp16 vaug [128, kt, 130]
             (two heads' 64+1 slots; ones col = softmax denominator).
  S^T      : fp8 DoubleRow, D=64 split across the two DR slots (d = 2p+i on 32
             partitions); 0.5 cycles/row.  q8/k8 reach [32,2,N] via an
             SBUF->SBUF DMA fold.
  exp      : two lanes, assigned per S-tile (pair of ktiles; GPSIMD cannot
             help: no PSUM port, and DMA cannot read PSUM either):
               ScalarE: true Exp -> fp16
               DVE:     Schraudolph magic in ONE tensor_scalar: uint16 out =
                        round(s*1024*log2(e)*scale + 15301.5), whose bit
                        pattern read as fp16 is ~exp(s*scale).
  AV       : q-major fp16-moving matmuls; per-qb sequential 16-step chains into
             one packed psum bank [128, 4, 65(pad 128)].
  norm     : DVE reciprocal + scalar_tensor_tensor broadcast-mult -> fp16 a2;
             aT via DMA xbar transpose (no PE, no psum).
  proj     : fp16 matmuls; ScalarE Identity applies 1/(SW*SP) + per-partition
             bias; yT [C, N] fp32 out (host transposes).
"""

import math

import numpy as np
import ml_dtypes

import concourse.bass as bass
import concourse.mybir as mybir
import concourse.tile as tile
from concourse import bacc
from concourse.bass_utils import run_bass_kernel_spmd

B, N, C, H = 8, 2048, 768, 12
D = C // H            # 64
CT = C // 128         # 6 channel tiles
NQ = 512              # query chunk (1 psum bank fp32)
NCH = N // NQ         # 4
NKT = N // 128        # 16 key tiles

SW = 32.0             # host scale on w_qkv
SP = 32.0             # host scale on w_proj
SCALE = float(D) ** -0.5
S_SCALE = SCALE / (SW * SW)        # exp scale on raw q8.k8 psum scores
LN2 = math.log(2.0)
A16 = 1024.0 * S_SCALE / LN2       # fp16 magic slope
B16 = 15301.5  # centered fp16 exponent bias (+.5 for trunc converts)

# Per-head-parity exp-lane assignment over the 8 S-tiles (pairs of ktiles).
# Act is the faster exp engine (0.83 vs 1.04 ns/row): give it the bigger share.
LANES = (
    "ADADADAA",   # head parity 0, chunk even
    "DADADAAD",   # head parity 1, chunk even
    "ADADDAAA",   # head parity 0, chunk odd
    "DADADAAD",   # head parity 1, chunk odd
)
# evac lane for q/k f-major downcasts and v downcasts, by slice parity
QK_EVAC = ("A", "D")
V_EVAC = ("D", "A")

FP32 = mybir.dt.float32
FP16 = mybir.dt.float16
F8 = mybir.dt.float8e4
EXP = mybir.ActivationFunctionType.Exp
IDENT = mybir.ActivationFunctionType.Identity
COPY = mybir.ActivationFunctionType.Copy
MULT = mybir.AluOpType.mult
ADD = mybir.AluOpType.add
DR = mybir.MatmulPerfMode.DoubleRow

F8NP = ml_dtypes.float8_e4m3

_CACHED_NC = None


def _ap(base, free_dims):
    """AP with base's partition dim and explicit [stride, count] free dims."""
    return bass.AP(
        tensor=base.tensor,
        offset=base.offset,
        ap=[list(base.ap[0])] + [list(d) for d in free_dims],
    )


def build():
    nc = bacc.Bacc()
    x_hi = nc.dram_tensor("x_hi", [128, CT, N], F8, kind="ExternalInput")
    x_lo = nc.dram_tensor("x_lo", [128, CT, N], F8, kind="ExternalInput")
    w_hi = nc.dram_tensor("w_hi", [128, CT, 3 * C], F8, kind="ExternalInput")
    w_lo = nc.dram_tensor("w_lo", [128, CT, 3 * C], F8, kind="ExternalInput")
    wp = nc.dram_tensor("wp", [128, CT, C], FP16, kind="ExternalInput")
    b2d = nc.dram_tensor("b2d", [128, CT], FP32, kind="ExternalInput")
    yT = nc.dram_tensor("yT", [C, N], FP32, kind="ExternalOutput")

    lp = nc.allow_low_precision("fp8/fp16 matmuls with fp32 psum accumulation")
    lp.__enter__()
    with tile.TileContext(nc) as tc:
        with tc.tile_pool(name="big", bufs=1) as big, \
             tc.tile_pool(name="fmp", bufs=2) as fmp, \
             tc.tile_pool(name="e16p", bufs=7) as e16p, \
             tc.tile_pool(name="e32p", bufs=8) as e32p, \
             tc.tile_pool(name="small", bufs=2) as small, \
             tc.tile_pool(name="ps2p", bufs=3, space="PSUM") as ps2p, \
             tc.tile_pool(name="psap", bufs=1, space="PSUM") as psap, \
             tc.tile_pool(name="psavp", bufs=1, space="PSUM") as psavp:

            # ---- persistent inputs -------------------------------------
            # loaded per ct-pair so the first DR chains start early
            xh = big.tile([128, CT, N], F8)
            xl = big.tile([128, CT, N], F8)
            wh = big.tile([128, CT, 3 * C], F8)
            wl = big.tile([128, CT, 3 * C], F8)
            wh16 = big.tile([128, CT, 3 * C], F8)
            xh16 = big.tile([128, CT, N], F8)
            for t in range(3):
                ts = slice(2 * t, 2 * t + 2)
                nc.sync.dma_start(out=wh[:, ts, :], in_=w_hi[:, ts, :])
                nc.sync.dma_start(out=xh[:, ts, :], in_=x_hi[:, ts, :])
                nc.sync.dma_start(out=xl[:, ts, :], in_=x_lo[:, ts, :])
                nc.sync.dma_start(out=wl[:, ts, :], in_=w_lo[:, ts, :])
            # derive the 1/16-prescaled operands on-chip (engines idle here)
            for t in range(3):
                ts = slice(2 * t, 2 * t + 2)
                nc.scalar.activation(out=wh16[:, ts, :], in_=wh[:, ts, :],
                                     func=COPY, scale=1.0 / 16.0)
                nc.vector.tensor_scalar(out=xh16[:, ts, :], in0=xh[:, ts, :],
                                        scalar1=1.0 / 16.0, scalar2=None,
                                        op0=MULT)
            wpt = big.tile([128, CT, C], FP16)
            nc.sync.dma_start(out=wpt, in_=wp[:, :, :])
            bias = big.tile([128, CT], FP32)
            nc.sync.dma_start(out=bias, in_=b2d[:, :])

            qg = [big.tile([128, 2, N], F8, name=f"qg{g}") for g in range(4)]
            kg = [big.tile([128, 2, N], F8, name=f"kg{g}") for g in range(4)]
            vaug = [
                big.tile([128, NKT, 130], FP16, name=f"vaug{p}") for p in range(6)
            ]
            for p in range(6):
                nc.gpsimd.memset(vaug[p][:, :, 64:65], 1.0)
                nc.gpsimd.memset(vaug[p][:, :, 129:130], 1.0)
            aT = big.tile([128, CT, N], FP16)

            # residual-DR chain: 9 matmuls into one psum, full precision.
            # pairs: (hi, hi), (hi16, lo), (lo16, hi) on (weights, x) — caller
            # passes the already-matched (lhsT, rhs) AP pairs.
            def dr_chain(ps_out, pairs):
                k = 0
                for (lt, lsl), (rt, rsl) in pairs:
                    for t in range(3):
                        nc.tensor.matmul(
                            ps_out,
                            lt[:, 2 * t:2 * t + 2, lsl],
                            rt[:, 2 * t:2 * t + 2, rsl],
                            start=(k == 0), stop=(k == 8), perf_mode=DR,
                        )
                        k += 1

            def evac(lane, out, in_):
                if lane == "A":
                    nc.scalar.activation(out=out, in_=in_, func=COPY, scale=1.0)
                else:
                    nc.vector.tensor_copy(out, in_)

            # ---- phase A pieces ----------------------------------------
            def emit_qk_side(hp, ch, side, fms):
                n0 = ch * NQ
                nsl = slice(n0, n0 + NQ)
                ps = psap.tile([128, NQ], FP32, tag="psa", name="psqk")
                f0 = 128 * (hp + 6 * side)
                fsl = slice(f0, f0 + 128)
                dr_chain(
                    ps[:, :],
                    (((wh, fsl), (xh, nsl)),
                     ((wh16, fsl), (xl, nsl)),
                     ((wl, fsl), (xh16, nsl))),
                )
                evac(QK_EVAC[(ch + side) % 2],
                     fms[side][:, n0:n0 + NQ], ps[:, :])

            def emit_v_half(hp, ch, j):
                # 2 key-tiles of this pair's V: kts {4ch+2j, 4ch+2j+1}
                vf0 = 1536 + 128 * hp
                vsl = slice(vf0, vf0 + 128)
                ps = psap.tile([128, NQ], FP32, tag="psa", name="psv")
                for i in (0, 1):
                    kt = 4 * ch + 2 * j + i
                    n0 = kt * 128
                    nsl = slice(n0, n0 + 128)
                    dr_chain(
                        ps[:, 128 * i:128 * i + 128],
                        (((xh, nsl), (wh, vsl)),
                         ((xl, nsl), (wh16, vsl)),
                         ((xh16, nsl), (wl, vsl))),
                    )
                vrow = vaug[hp][:, 4 * ch + 2 * j, :]
                out = _ap(vrow, [[130, 2], [65, 2], [1, 64]])
                src = _ap(ps[:, :], [[128, 2], [64, 2], [1, 64]])
                evac(V_EVAC[(ch + j) % 2], out, src)

            def new_fm():
                fq = fmp.tile([128, N], F8, tag="fmq", name="fq")
                fk = fmp.tile([128, N], F8, tag="fmk", name="fk")
                return fq, fk

            def emit_a_slice(hp, ch, fms):
                emit_qk_side(hp, ch, 0, fms)
                emit_v_half(hp, ch, 0)
                emit_qk_side(hp, ch, 1, fms)
                emit_v_half(hp, ch, 1)

            def emit_rearrange_ch(hp, ch, fms):
                # fold [64, NQ] f-major chunk into [32, 2, NQ] (d = 2p + i)
                n0 = ch * NQ
                for side, grps in ((0, qg), (1, kg)):
                    for hi in (0, 1):
                        h = 2 * hp + hi
                        g, q4 = h // 3, h % 3
                        nc.sync.dma_start(
                            out=grps[g][32 * q4:32 * q4 + 32, :, n0:n0 + NQ],
                            in_=fms[side][64 * hi:64 * hi + 64, n0:n0 + NQ],
                        )

            # ---- phase B: S+exp for head k runs while head k-1's AV/norm
            # retires, so the exp engines always have fresh psums ------------
            def emit_s_exp(hp, ch, hi):
                n0 = ch * NQ
                h = 2 * hp + hi
                g, q4 = h // 3, h % 3
                p0 = 32 * q4
                qs = qg[g][p0:p0 + 32, :, n0:n0 + NQ]
                lanes = LANES[hi + 2 * (ch % 2)]
                ets = []
                for st in range(8):
                    ps = ps2p.tile([128, 2, NQ], FP32, tag="ps2", name="sps")
                    for tt in (0, 1):
                        kt = 2 * st + tt
                        nc.tensor.matmul(
                            ps[:, tt, :],
                            kg[g][p0:p0 + 32, :, kt * 128:kt * 128 + 128],
                            qs,
                            start=True, stop=True, perf_mode=DR,
                        )
                    if lanes[st] == "A":
                        et = e16p.tile([128, 2, NQ], FP16, tag="e16", name="e16")
                        nc.scalar.activation(
                            out=et, in_=ps, func=EXP, scale=S_SCALE
                        )
                        ets.append((et, False))
                    else:
                        # fp32 -> uint16 convert IS the magic: i16 lands as the
                        # fp16 bit pattern of ~exp(s)
                        et = e32p.tile([128, 2, NQ], mybir.dt.uint16,
                                       tag="e32", name="e32")
                        nc.vector.tensor_scalar(
                            out=et, in0=ps, scalar1=A16, scalar2=B16,
                            op0=MULT, op1=ADD,
                        )
                        ets.append((et, True))
                return (hp, ch, hi, ets)

            a2map = {}

            def emit_av_norm(ctx):
                hp, ch, hi, ets = ctx
                n0 = ch * NQ
                if hi == 0:
                    a2map[(hp, ch)] = small.tile(
                        [128, 4, 128], FP16, tag="a2", name="a2", bufs=2
                    )
                a2 = a2map[(hp, ch)]
                av = psavp.tile([128, 4, 128], FP32, tag="av", name="av")
                for qb in range(4):
                    for st in range(8):
                        et, magic = ets[st]
                        for tt in (0, 1):
                            kt = 2 * st + tt
                            if magic:
                                l = et.bitcast(FP16)[:, tt, qb * 128:qb * 128 + 128]
                            else:
                                l = et[:, tt, qb * 128:qb * 128 + 128]
                            nc.tensor.matmul(
                                av[:, qb, 0:65],
                                l,
                                vaug[hp][:, kt, 65 * hi:65 * hi + 65],
                                start=(st == 0 and tt == 0),
                                stop=(st == 7 and tt == 1),
                            )
                recp = small.tile([128, 4], FP32, tag="recp", name="recp")
                nc.vector.reciprocal(recp, av[:, :, 64])
                nc.vector.scalar_tensor_tensor(
                    out=a2[:, :, 64 * hi:64 * hi + 64],
                    in0=av[:, :, 0:64],
                    scalar=1.0,
                    in1=_ap(recp, [[1, 4], [0, 64]]),
                    op0=MULT, op1=MULT,
                )
                if hi == 1:
                    nc.sync.dma_start_transpose(
                        out=aT[:, hp, n0:n0 + NQ].rearrange(
                            "p (qb q) -> p qb q", qb=4
                        ),
                        in_=a2.rearrange("p qb d -> p (qb d)"),
                    )
                    del a2map[(hp, ch)]

            # ---- phase C ------------------------------------------------
            def emit_c():
                for ot in range(6):
                    for ch in range(NCH):
                        n0 = ch * NQ
                        ps = ps2p.tile([128, 2, NQ], FP32, tag="ps2", name="cps")
                        for ct in range(CT):
                            nc.tensor.matmul(
                                ps[:, 0, :],
                                wpt[:, ct, 128 * ot:128 * ot + 128],
                                aT[:, ct, n0:n0 + NQ],
                                start=(ct == 0), stop=(ct == CT - 1),
                            )
                        ysb = e32p.tile([128, NQ], FP32, tag="e32", name="ysb")
                        nc.scalar.activation(
                            out=ysb, in_=ps[:, 0, :], func=IDENT,
                            scale=1.0 / (SW * SP), bias=bias[:, ot:ot + 1],
                        )
                        nc.sync.dma_start(
                            out=yT[128 * ot:128 * ot + 128, n0:n0 + NQ], in_=ysb
                        )

            # ---- emission: A slices pipelined one pair ahead of B, AV
            # blocks deferred one head behind their S+exp ------------------
            fms = new_fm()
            for ch in range(NCH):
                emit_a_slice(0, ch, fms)
                emit_rearrange_ch(0, ch, fms)
            pend = None
            for hp in range(6):
                nfms = new_fm() if hp < 5 else None
                for ch in range(NCH):
                    for hi in (0, 1):
                        ctx = emit_s_exp(hp, ch, hi)
                        if pend is not None:
                            emit_av_norm(pend)
                        pend = ctx
                        if hp < 5:
                            emit_qk_side(hp + 1, ch, hi, nfms)
                            emit_v_half(hp + 1, ch, hi)
                            if hi == 1:
                                emit_rearrange_ch(hp + 1, ch, nfms)
            emit_av_norm(pend)
            emit_c()
    lp.__exit__(None, None, None)

    nc.finalize()
    return nc


def get_nc():
    global _CACHED_NC
    if _CACHED_NC is None:
        _CACHED_NC = build()
    return _CACHED_NC


def _prep_shared(w_qkv, w_proj, b_proj):
    wq = (w_qkv.astype(np.float64) * SW).astype(np.float32)
    w_hi = wq.astype(F8NP)
    w_lo = ((wq - w_hi.astype(np.float32)) * 16.0).astype(F8NP)

    def lay_w(a):
        return np.ascontiguousarray(a.T.reshape(CT, 128, 3 * C).transpose(1, 0, 2))
    wpm = (w_proj.astype(np.float64) * SP).astype(np.float16)
    wp_l = np.ascontiguousarray(wpm.T.reshape(CT, 128, C).transpose(1, 0, 2))
    b2d = np.ascontiguousarray(b_proj.reshape(CT, 128).T.astype(np.float32))
    return lay_w(w_hi), lay_w(w_lo), wp_l, b2d


def _prep_x(xi):
    xs = np.ascontiguousarray(xi.T.reshape(CT, 128, N).transpose(1, 0, 2))
    x_hi = xs.astype(F8NP)
    x_lo = ((xs - x_hi.astype(np.float32)) * 16.0).astype(F8NP)
    return x_hi, x_lo


LAST_RESULT = None


def kernel(x, w_qkv, w_proj, b_proj, **run_kwargs):
    x = np.ascontiguousarray(np.asarray(x, dtype=np.float32))
    w_qkv = np.ascontiguousarray(np.asarray(w_qkv, dtype=np.float32))
    w_proj = np.ascontiguousarray(np.asarray(w_proj, dtype=np.float32))
    b_proj = np.ascontiguousarray(np.asarray(b_proj, dtype=np.float32))
    assert x.shape == (B, N, C)

    nc = get_nc()
    w_hi, w_lo_l, wp_l, b2d = _prep_shared(w_qkv, w_proj, b_proj)
    in_maps = []
    for i in range(B):
        x_hi, x_lo = _prep_x(x[i])
        in_maps.append({
            "x_hi": x_hi, "x_lo": x_lo,
            "w_hi": w_hi, "w_lo": w_lo_l,
            "wp": wp_l, "b2d": b2d,
        })
    res = run_bass_kernel_spmd(nc, in_maps, list(range(B)), **run_kwargs)
    global LAST_RESULT
    LAST_RESULT = res
    out = np.stack(
        [np.ascontiguousarray(res.results[i]["yT"].T) for i in range(B)], axis=0
    )
    return out


if __name__ == "__main__":
    rng = np.random.default_rng(0)
    x = rng.standard_normal((B, N, C), dtype=np.float32)
    w_qkv = (rng.standard_normal((3 * C, C)) * 0.02).astype(np.float32)
    w_proj = (rng.standard_normal((C, C)) * 0.02).astype(np.float32)
    b_proj = (rng.standard_normal((C,)) * 0.02).astype(np.float32)
    out = kernel(x=x, w_qkv=w_qkv, w_proj=w_proj, b_proj=b_proj)
    print("out", out.shape, out.dtype, float(np.abs(out).max()))
